# revision 1
# baseline (speedup 1.0000x reference)
"""Trainium2 Bass kernel for a dense transformer block (pre-LN, causal MHA + FFN).

Sharding (8 NeuronCores): core c = 2*b + g handles sequence b (of B=4) and
half g (of 2): tensor-parallel attention over 8 of 16 heads (partial proj,
pairwise ReduceScatter over {2b, 2b+1}), then token-parallel LN2+FFN over
its 1024 of 2048 tokens. Device kernel works in transposed [C, T] layout;
host transposes in/out.

LayerNorm is folded into the matmuls: for Q^T = Wq^T @ LN(x)^T we accumulate
M = W~^T x plus rank-1 corrections (colsum(W~) (x) -mu + (beta@W) (x) std)
in PSUM, then scale columns by rstd at eviction. Same trick for the FFN
(relu is positively homogeneous, so rstd2 commutes out to the ff2 evict).

Matmul dtypes: float32r (full-rate fp32 variant, fed via casting DMAs) for
QKV/S/FFN-1; bf16 for attention V*P, proj, and FFN-2.
"""
import numpy as np
import ml_dtypes
from contextlib import ExitStack

B, T, C = 4, 2048, 1024
H, HS = 16, 64
F = 4 * C
P = 128
EPS = 1e-5
NCT = C // P        # 8 c-tiles
NFT = F // P        # 32 f-tiles
TL = T // 2         # 1024 local tokens
NPAIR = 4           # head-pairs per core
GROUPS = [[0, 1], [2, 3], [4, 5], [6, 7]]

_CACHE = {}


def _build(with_collective=True):
    import concourse.tile as tile
    from concourse import bacc, mybir

    f32 = mybir.dt.float32
    f32r = mybir.dt.float32r
    bf16 = mybir.dt.bfloat16
    AF = mybir.ActivationFunctionType
    OP = mybir.AluOpType

    nc = bacc.Bacc("TRN2", target_bir_lowering=False, debug=False, num_devices=8)

    # ---- DRAM I/O ----
    d_xT = nc.dram_tensor("xT", [NCT, P, T], f32, kind="ExternalInput").ap()
    d_xres = nc.dram_tensor("xresT", [NCT, P, TL], f32, kind="ExternalInput").ap()
    d_wqkv = nc.dram_tensor("wqkv", [NPAIR, NCT, P, 3 * P], f32,
                            kind="ExternalInput").ap()
    d_ccqkv = nc.dram_tensor("ccqkv", [NPAIR, 2, 3 * P], f32,
                             kind="ExternalInput").ap()
    d_wproj = nc.dram_tensor("wproj", [NPAIR, P, C], bf16, kind="ExternalInput").ap()
    d_bproj = nc.dram_tensor("bproj", [NCT, P], f32, kind="ExternalInput").ap()
    d_w1 = nc.dram_tensor("w1", [NCT, P, F], bf16, kind="ExternalInput").ap()
    d_ccf = nc.dram_tensor("ccf", [NFT // 4, 2, 512], bf16, kind="ExternalInput").ap()
    d_w2 = nc.dram_tensor("w2", [NCT, NFT // 4, P, 512], bf16, kind="ExternalInput").ap()
    d_b2 = nc.dram_tensor("b2", [NCT, P], f32, kind="ExternalInput").ap()
    d_m01 = nc.dram_tensor("m01", [4, P, 512], bf16, kind="ExternalInput").ap()
    d_ident = nc.dram_tensor("ident", [P, P], f32, kind="ExternalInput").ap()
    d_out = nc.dram_tensor("outT", [NCT, P, TL], f32, kind="ExternalOutput").ap()

    with tile.TileContext(nc) as tc, ExitStack() as ctx:
        dram = ctx.enter_context(tc.tile_pool(name="dram", bufs=1, space="DRAM"))
        sa_bounce = [dram.tile([2, NCT // 2, P, TL], f32, name=f"sab{h}")
                     for h in range(2)]
        sa_local = [dram.tile([NCT // 2, P, TL], f32, name=f"sal{h}")
                    for h in range(2)]

        const = ctx.enter_context(tc.tile_pool(name="const", bufs=1))
        ones_bf = const.tile([P, 1], bf16)
        nc.vector.memset(ones_bf[:], 1.0)
        ident_f32 = const.tile([P, P], f32)
        nc.sync.dma_start(ident_f32[:], d_ident[:])
        masks = [const.tile([P, 512], bf16, name=f"mask{i}", tag=f"mask{i}")
                 for i in range(4)]
        for i in range(4):
            nc.sync.dma_start(masks[i][:], d_m01[i])
        onescol = const.tile([1, P], f32)
        nc.vector.memset(onescol[:], 1.0)

        x2_dram = dram.tile([NCT, P, TL], f32)

        # long-lived pools, first-use pinned bottom-up so frees are LIFO.
        abc_pool = ctx.enter_context(tc.tile_pool(name="abc", bufs=1))
        a1bc = [abc_pool.tile([P, 512], f32, name=f"a1bc{ch}", tag=f"a1bc{ch}")
                for ch in range(4)]
        a2bc = [abc_pool.tile([P, 512], f32, name=f"a2bc{ch}", tag=f"a2bc{ch}")
                for ch in range(2)]
        for t_ in a1bc + a2bc:
            nc.vector.memset(t_[:, 0:1], 0.0)  # pin allocation order
        rowr_pool = ctx.enter_context(tc.tile_pool(name="rowr", bufs=1))
        rowr_pin = rowr_pool.tile([1, 1], f32, tag="pin")
        nc.vector.memset(rowr_pin[:], 0.0)
        pattr = ExitStack()
        attT_pool = pattr.enter_context(tc.tile_pool(name="attT", bufs=1))
        attT = [attT_pool.tile([P, T], bf16, name=f"attT{p}", tag=f"attT{p}")
                for p in range(NPAIR)]
        for p in range(NPAIR):
            nc.vector.memset(attT[p][:, 0:1], 0.0)
        pqkv = ExitStack()
        vaug_pool = pqkv.enter_context(tc.tile_pool(name="vaug", bufs=1))
        qq_pool = pqkv.enter_context(tc.tile_pool(name="qq", bufs=1))
        kk_pool = pqkv.enter_context(tc.tile_pool(name="kk", bufs=1))
        v_aug = {}
        for p in range(NPAIR):
            for st in range(16):
                va = vaug_pool.tile([P, 130], bf16, name=f"va{p}_{st}",
                                    tag=f"va{p}_{st}")
                nc.vector.memset(va[:, 64:65], 1.0)
                nc.vector.memset(va[:, 129:130], 1.0)
                v_aug[(p, st)] = va
        qq_r = [qq_pool.tile([P, T], bf16, name=f"qq{p}", tag=f"qq{p}")
                for p in range(NPAIR)]
        kk_r = [kk_pool.tile([P, T], bf16, name=f"kk{p}", tag=f"kk{p}")
                for p in range(NPAIR)]
        for p in range(NPAIR):
            nc.vector.memset(qq_r[p][:, 0:1], 0.0)
            nc.vector.memset(kk_r[p][:, 0:1], 0.0)

        # =========== Phase 1: LN1 stats ===========
        p1 = ExitStack()
        xc_pool = p1.enter_context(tc.tile_pool(name="xc", bufs=6))
        bfc_pool = p1.enter_context(tc.tile_pool(name="bfc", bufs=8))
        rows1_pool = p1.enter_context(tc.tile_pool(name="rows1", bufs=6))
        stat_ps = p1.enter_context(tc.tile_pool(name="statps", bufs=2, space="PSUM"))
        bcp_ps = p1.enter_context(tc.tile_pool(name="bcpps", bufs=2, space="PSUM"))

        mu_row = rows1_pool.tile([1, T], f32, tag="row")
        ex2_row = rows1_pool.tile([1, T], f32, tag="row")
        var_row = rows1_pool.tile([1, T], f32, tag="row")
        std_row = rows1_pool.tile([1, T], f32, tag="row")
        rstd_row = rows1_pool.tile([1, T], f32, tag="row")
        nm_row = rows1_pool.tile([1, T], f32, tag="row")
        xrow1_r = rowr_pool.tile([2, T], f32r, tag="xrowr")
        for ch in range(4):
            sl = slice(ch * 512, (ch + 1) * 512)
            sx_ps = stat_ps.tile([1, 512], f32, tag="sx")
            sq_ps = stat_ps.tile([1, 512], f32, tag="sq")
            for ci in range(NCT):
                xc = xc_pool.tile([P, 512], f32, tag="xc")
                nc.sync.dma_start(xc[:], d_xT[ci][:, sl])
                xbfc = bfc_pool.tile([P, 512], bf16, tag="xbfc")
                nc.vector.tensor_copy(xbfc[:], xc[:])
                sqc = bfc_pool.tile([P, 512], bf16, tag="sqc")
                nc.vector.tensor_mul(sqc[:], xc[:], xc[:])
                nc.tensor.matmul(sx_ps[:], ones_bf[:], xbfc[:],
                                 start=(ci == 0), stop=(ci == NCT - 1))
                nc.tensor.matmul(sq_ps[:], ones_bf[:], sqc[:],
                                 start=(ci == 0), stop=(ci == NCT - 1))
            # per-chunk stats math so QKV corrections unblock early
            nc.scalar.mul(mu_row[:, sl], sx_ps[:], 1.0 / C)
            nc.scalar.mul(ex2_row[:, sl], sq_ps[:], 1.0 / C)
            nc.vector.tensor_mul(var_row[:, sl], mu_row[:, sl], mu_row[:, sl])
            nc.vector.scalar_tensor_tensor(var_row[:, sl], ex2_row[:, sl], EPS,
                                           var_row[:, sl], OP.add, OP.subtract)
            nc.scalar.activation(std_row[:, sl], var_row[:, sl], AF.Sqrt)
            nc.vector.reciprocal(rstd_row[:, sl], std_row[:, sl])
            nc.scalar.mul(nm_row[:, sl], mu_row[:, sl], -1.0)
            nc.gpsimd.dma_start(xrow1_r[0:1, sl], nm_row[:, sl])
            nc.gpsimd.dma_start(xrow1_r[1:2, sl], std_row[:, sl])
            bc_ps = bcp_ps.tile([P, 512], f32, tag="bc")
            nc.tensor.matmul(bc_ps[:], onescol[:], rstd_row[:, sl],
                             start=True, stop=True)
            nc.scalar.copy(a1bc[ch][:], bc_ps[:])
        p1.close()

        # ===== Phases 2+3 (interleaved per pair): QKV + attention =====
        pat = ExitStack()
        w_pool = pat.enter_context(tc.tile_pool(name="wqkv", bufs=12))
        cc_pool = pat.enter_context(tc.tile_pool(name="cc", bufs=5))
        xr_pool = pat.enter_context(tc.tile_pool(name="xr", bufs=1))
        ev_pool = pat.enter_context(tc.tile_pool(name="ev", bufs=3))
        e_pool = pat.enter_context(tc.tile_pool(name="epool", bufs=8))
        rec_pool = pat.enter_context(tc.tile_pool(name="rec", bufs=2))
        bcsb_pool = pat.enter_context(tc.tile_pool(name="bcsb", bufs=2))
        mps = pat.enter_context(tc.tile_pool(name="mps", bufs=1, space="PSUM"))

        xr = [xr_pool.tile([P, T], f32r, name=f"xr{ci}", tag=f"xr{ci}")
              for ci in range(NCT)]
        for ci in range(NCT):
            nc.gpsimd.dma_start(xr[ci][:], d_xT[ci])

        for p in range(NPAIR):
            # --- QKV for pair p (LN folded via corrections + rstd evict scale) ---
            w_t = []
            for ci in range(NCT):
                w = w_pool.tile([P, 3 * P], f32r, tag="w")
                nc.gpsimd.dma_start(w[:], d_wqkv[p, ci])
                w_t.append(w)
            cc = cc_pool.tile([2, 3 * P], f32r, tag="cc")
            nc.gpsimd.dma_start(cc[:], d_ccqkv[p])
            for ch in range(4):
                sl = slice(ch * 512, (ch + 1) * 512)
                q_ps = mps.tile([P, 512], f32, tag="q_ps")
                k_ps = mps.tile([P, 512], f32, tag="k_ps")
                v_ps = mps.tile([P, 512], f32, tag="v_ps")
                for ci in range(NCT):
                    nc.tensor.matmul(q_ps[:], w_t[ci][:, 0:P], xr[ci][:, sl],
                                     start=(ci == 0), stop=False)
                    nc.tensor.matmul(k_ps[:], w_t[ci][:, P:2 * P], xr[ci][:, sl],
                                     start=(ci == 0), stop=False)
                    nc.tensor.matmul(v_ps[:], w_t[ci][:, 2 * P:3 * P], xr[ci][:, sl],
                                     start=(ci == 0), stop=False)
                nc.tensor.matmul(q_ps[:], cc[:, 0:P], xrow1_r[:, sl],
                                 start=False, stop=True)
                nc.tensor.matmul(k_ps[:], cc[:, P:2 * P], xrow1_r[:, sl],
                                 start=False, stop=True)
                nc.tensor.matmul(v_ps[:], cc[:, 2 * P:3 * P], xrow1_r[:, sl],
                                 start=False, stop=True)
                nc.vector.tensor_mul(qq_r[p][:, sl], q_ps[:], a1bc[ch][:])
                nc.vector.tensor_mul(kk_r[p][:, sl], k_ps[:], a1bc[ch][:])
                vev = ev_pool.tile([P, 512], f32, tag="vev")
                nc.vector.tensor_mul(vev[:], v_ps[:], a1bc[ch][:])
                for sti in range(4):
                    st = ch * 4 + sti
                    for hh in range(2):
                        tp = mps.tile([P, 64], f32, tag="tr")
                        nc.tensor.transpose(
                            tp[:], vev[hh * 64:(hh + 1) * 64, sti * 128:(sti + 1) * 128],
                            ident_f32[hh * 64:(hh + 1) * 64, hh * 64:(hh + 1) * 64])
                        nc.vector.tensor_copy(v_aug[(p, st)][:, hh * 65:hh * 65 + 64],
                                              tp[:])

            # --- attention for pair p (row-packed S^T, aug-row denominators) ---
            for qc in range(4):
                qsl = slice(qc * 512, (qc + 1) * 512)
                n_st = 4 * (qc + 1)
                attA = mps.tile([65, 512], f32, tag="attA")
                attB = mps.tile([65, 512], f32, tag="attB")
                for si in range(n_st):
                    ssl = slice(si * 128, (si + 1) * 128)
                    stA = mps.tile([P, 512], f32, tag="stA")
                    stB = mps.tile([P, 512], f32, tag="stB")
                    nc.tensor.matmul(stA[:], kk_r[p][0:64, ssl], qq_r[p][0:64, qsl],
                                     start=True, stop=True)
                    nc.tensor.matmul(stB[:], kk_r[p][64:128, ssl], qq_r[p][64:128, qsl],
                                     start=True, stop=True)
                    eA = e_pool.tile([P, 512], bf16, tag="e")
                    eB = e_pool.tile([P, 512], bf16, tag="e")
                    nc.scalar.activation(eA[:], stA[:], AF.Exp)
                    nc.scalar.activation(eB[:], stB[:], AF.Exp)
                    if si >= 4 * qc:
                        off = si - 4 * qc
                        nc.vector.tensor_mul(eA[:], eA[:], masks[off][:])
                        nc.vector.tensor_mul(eB[:], eB[:], masks[off][:])
                    nc.tensor.matmul(attA[:], v_aug[(p, si)][:, 0:65], eA[:],
                                     start=(si == 0), stop=(si == n_st - 1))
                    nc.tensor.matmul(attB[:], v_aug[(p, si)][:, 65:130], eB[:],
                                     start=(si == 0), stop=(si == n_st - 1))
                for hh, att in ((0, attA), (1, attB)):
                    rec = rec_pool.tile([1, 512], f32, tag="rec")
                    nc.vector.reciprocal(rec[:], att[64:65, :])
                    bc_sb = bcsb_pool.tile([64, 512], f32, tag="bc_sb")
                    nc.gpsimd.partition_broadcast(bc_sb[:], rec[:])
                    nc.vector.tensor_mul(attT[p][hh * 64:(hh + 1) * 64, qsl],
                                         att[0:64, :], bc_sb[:])
        pat.close()
        pqkv.close()

        # ===== Phase 4: proj -> ReduceScatter -> x2 (+ fused LN2 stats) =====
        px2bf = ExitStack()
        x2bf_pool = px2bf.enter_context(tc.tile_pool(name="x2bf", bufs=1))
        x2bf = [x2bf_pool.tile([P, TL], bf16, name=f"x2bf{ci}", tag=f"x2bf{ci}")
                for ci in range(NCT)]
        for ci in range(NCT):
            nc.vector.memset(x2bf[ci][:, 0:1], 0.0)  # pin allocation order

        p4 = ExitStack()
        wp_pool = p4.enter_context(tc.tile_pool(name="wproj", bufs=1))
        proj_ps = p4.enter_context(tc.tile_pool(name="projps", bufs=2, space="PSUM"))
        sa_pool = p4.enter_context(tc.tile_pool(name="sasb", bufs=4))
        xres_pool = p4.enter_context(tc.tile_pool(name="xres", bufs=2))
        bpj_pool = p4.enter_context(tc.tile_pool(name="bpj", bufs=1))
        sq_pool = p4.enter_context(tc.tile_pool(name="sq2", bufs=3))
        rows2_pool = p4.enter_context(tc.tile_pool(name="rows2", bufs=4))
        stat_ps2 = p4.enter_context(tc.tile_pool(name="statps2", bufs=2, space="PSUM"))
        bcp_ps2 = p4.enter_context(tc.tile_pool(name="bcpps2", bufs=1, space="PSUM"))

        bprojcol = [bpj_pool.tile([P, 1], f32, name=f"bpj{ci}", tag=f"bpj{ci}")
                    for ci in range(NCT)]
        for ci in range(NCT):
            nc.sync.dma_start(bprojcol[ci][:], d_bproj[ci].unsqueeze(1))
        wp = []
        for ki in range(NPAIR):
            w = wp_pool.tile([P, C], bf16, name=f"wp{ki}", tag=f"wp{ki}")
            nc.sync.dma_start(w[:], d_wproj[ki])
            wp.append(w)
        for co in range(NCT):
            for tc4 in range(4):
                sl = slice(tc4 * 512, (tc4 + 1) * 512)
                pp = proj_ps.tile([P, 512], f32, tag="pp")
                for ki in range(NPAIR):
                    nc.tensor.matmul(pp[:], wp[ki][:, co * P:(co + 1) * P],
                                     attT[ki][:, sl],
                                     start=(ki == 0), stop=(ki == NPAIR - 1))
                sa_sb = sa_pool.tile([P, 512], f32, tag="sa_sb")
                nc.vector.tensor_copy(sa_sb[:], pp[:])
                fold, off = tc4 // 2, (tc4 % 2) * 512
                nc.sync.dma_start(
                    sa_bounce[co // 4][fold, co % 4, :, off:off + 512], sa_sb[:])
            if co == 3 or co == NCT - 1:
                h = co // 4
                if with_collective:
                    nc.gpsimd.collective_compute(
                        "ReduceScatter",
                        OP.add,
                        replica_groups=GROUPS,
                        ins=[sa_bounce[h].opt()],
                        outs=[sa_local[h].opt()],
                    )
                else:
                    nc.sync.dma_start(sa_local[h][:], sa_bounce[h][0])

        sx_ch = [stat_ps2.tile([1, 512], f32, name=f"sx2_{ch}", tag="sx")
                 for ch in range(2)]
        sq_ch = [stat_ps2.tile([1, 512], f32, name=f"sq2_{ch}", tag="sq")
                 for ch in range(2)]
        for co in range(NCT):
            sal = xres_pool.tile([P, TL], f32, tag="sal")
            nc.sync.dma_start(sal[:], sa_local[co // 4][co % 4])
            xres = xres_pool.tile([P, TL], f32, tag="xres")
            nc.sync.dma_start(xres[:], d_xres[co])
            x2sb = xres_pool.tile([P, TL], f32, tag="x2sb")
            nc.vector.scalar_tensor_tensor(x2sb[:], sal[:], bprojcol[co][:],
                                           xres[:], OP.add, OP.add)
            nc.sync.dma_start(x2_dram[co], x2sb[:])
            nc.vector.tensor_copy(x2bf[co][:], x2sb[:])
            sqt = sq_pool.tile([P, TL], bf16, tag="sqt")
            nc.vector.tensor_mul(sqt[:], x2sb[:], x2sb[:])
            for ch in range(2):
                sl = slice(ch * 512, (ch + 1) * 512)
                nc.tensor.matmul(sx_ch[ch][:], ones_bf[:], x2bf[co][:, sl],
                                 start=(co == 0), stop=(co == NCT - 1))
                nc.tensor.matmul(sq_ch[ch][:], ones_bf[:], sqt[:, sl],
                                 start=(co == 0), stop=(co == NCT - 1))

        mu2 = rows2_pool.tile([1, TL], f32, tag="row")
        ex22 = rows2_pool.tile([1, TL], f32, tag="row")
        for ch in range(2):
            sl = slice(ch * 512, (ch + 1) * 512)
            nc.scalar.mul(mu2[:, sl], sx_ch[ch][:], 1.0 / C)
            nc.scalar.mul(ex22[:, sl], sq_ch[ch][:], 1.0 / C)
        var2 = rows2_pool.tile([1, TL], f32, tag="row")
        nc.vector.tensor_mul(var2[:], mu2[:], mu2[:])
        nc.vector.scalar_tensor_tensor(var2[:], ex22[:], EPS,
                                       var2[:], OP.add, OP.subtract)
        std2 = rows2_pool.tile([1, TL], f32, tag="row")
        nc.scalar.activation(std2[:], var2[:], AF.Sqrt)
        rstd2 = rows2_pool.tile([1, TL], f32, tag="row")
        nc.vector.reciprocal(rstd2[:], std2[:])
        nm2 = rows2_pool.tile([1, TL], f32, tag="row")
        nc.scalar.mul(nm2[:], mu2[:], -1.0)

        xrow2_r = rowr_pool.tile([2, TL], bf16, tag="xrow2b")
        nc.gpsimd.dma_start(xrow2_r[0:1, :], nm2[:])
        nc.gpsimd.dma_start(xrow2_r[1:2, :], std2[:])
        for ch in range(2):
            sl = slice(ch * 512, (ch + 1) * 512)
            bc_ps = bcp_ps2.tile([P, 512], f32, tag="bc")
            nc.tensor.matmul(bc_ps[:], onescol[:], rstd2[:, sl],
                             start=True, stop=True)
            nc.scalar.copy(a2bc[ch][:], bc_ps[:])
        p4.close()

        # =========== Phase 6: FFN (LN folded, rstd2 deferred to ff2 evict) ===========
        p6 = ExitStack()
        w1_pool = p6.enter_context(tc.tile_pool(name="w1", bufs=16))
        ccf_pool = p6.enter_context(tc.tile_pool(name="ccf", bufs=4))
        w2_pool = p6.enter_context(tc.tile_pool(name="w2", bufs=16))
        ffn_ps = p6.enter_context(tc.tile_pool(name="ffnps", bufs=3, space="PSUM"))
        relu_pool = p6.enter_context(tc.tile_pool(name="relu", bufs=1))
        out_pool = p6.enter_context(tc.tile_pool(name="outsb", bufs=4))
        b2_pool = p6.enter_context(tc.tile_pool(name="b2p", bufs=1))

        b2col = [b2_pool.tile([P, 1], f32, name=f"b2c{ci}", tag=f"b2c{ci}")
                 for ci in range(NCT)]
        for ci in range(NCT):
            nc.sync.dma_start(b2col[ci][:], d_b2[ci].unsqueeze(1))

        relu1 = []
        for fog in range(NFT // 4):
            w1t = []
            for ci in range(NCT):
                w = w1_pool.tile([P, 512], bf16, tag="w1t")
                nc.sync.dma_start(w[:], d_w1[ci][:, fog * 512:(fog + 1) * 512])
                w1t.append(w)
            ccf = ccf_pool.tile([2, 512], bf16, tag="ccf")
            nc.sync.dma_start(ccf[:], d_ccf[fog])
            for fol in range(4):
                fo = fog * 4 + fol
                fsl = slice(fol * P, (fol + 1) * P)
                rt = relu_pool.tile([P, TL], bf16, name=f"rl{fo}", tag=f"rl{fo}")
                relu1.append(rt)
                for tc2 in range(2):
                    sl = slice(tc2 * 512, (tc2 + 1) * 512)
                    fp = ffn_ps.tile([P, 512], f32, tag="fp")
                    for ci in range(NCT):
                        nc.tensor.matmul(fp[:], w1t[ci][:, fsl], x2bf[ci][:, sl],
                                         start=(ci == 0), stop=False)
                    nc.tensor.matmul(fp[:], ccf[:, fsl], xrow2_r[:, sl],
                                     start=False, stop=True)
                    nc.scalar.activation(rt[:, sl], fp[:], AF.Relu)

        for co in range(NCT):
            w2g = []
            for gq in range(NFT // 4):
                w = w2_pool.tile([P, 512], bf16, tag="w2t")
                nc.sync.dma_start(w[:], d_w2[co, gq])
                w2g.append(w)
            for tc2 in range(2):
                sl = slice(tc2 * 512, (tc2 + 1) * 512)
                fp = ffn_ps.tile([P, 512], f32, tag="fp")
                for fi in range(NFT):
                    nc.tensor.matmul(fp[:], w2g[fi // 4][:, (fi % 4) * P:(fi % 4 + 1) * P],
                                     relu1[fi][:, sl],
                                     start=(fi == 0), stop=(fi == NFT - 1))
                x2c = out_pool.tile([P, 512], f32, tag="x2c")
                nc.sync.dma_start(x2c[:], x2_dram[co][:, sl])
                tmp = out_pool.tile([P, 512], f32, tag="tmp")
                nc.vector.tensor_mul(tmp[:], fp[:], a2bc[tc2][:])
                osb = out_pool.tile([P, 512], f32, tag="osb")
                nc.vector.scalar_tensor_tensor(osb[:], tmp[:], b2col[co][:],
                                               x2c[:], OP.add, OP.add)
                nc.sync.dma_start(d_out[co][:, sl], osb[:])
        p6.close()
        px2bf.close()
        pattr.close()

    nc.compile()
    return nc


def _prep_inputs(x, Wq, Wk, Wv, Wproj, bproj, W1, b1, W2, b2, g1, beta1, g2, beta2):
    """Build the 8 per-core input maps (host-side sharding + layout prep)."""
    f32 = np.float32
    scale = HS ** -0.5
    x = np.asarray(x, f32)
    Wq = np.asarray(Wq, f32)
    Wk = np.asarray(Wk, f32)
    Wv = np.asarray(Wv, f32)
    Wproj = np.asarray(Wproj, f32)
    W1 = np.asarray(W1, f32)
    b1 = np.asarray(b1, f32)
    W2 = np.asarray(W2, f32)
    g1 = np.asarray(g1, f32)
    beta1 = np.asarray(beta1, f32)
    g2 = np.asarray(g2, f32)
    beta2 = np.asarray(beta2, f32)

    w1g = g2[:, None] * W1
    b1p = b1 + beta2 @ W1
    w1bf = w1g.reshape(NCT, P, F).astype(ml_dtypes.bfloat16)
    ccfbf = np.stack([w1g.sum(0).reshape(NFT // 4, 512),
                      b1p.reshape(NFT // 4, 512)], axis=1).astype(ml_dtypes.bfloat16)
    # [NCT, NFT//4, P, 512]: w2h[co, gq, p, fi*128+cc] = W2[(gq*4+fi)*128+p, co*128+cc]
    w2bf = np.ascontiguousarray(
        W2.reshape(NFT // 4, 4, P, NCT, P).transpose(3, 0, 2, 1, 4).reshape(
            NCT, NFT // 4, P, 512)).astype(ml_dtypes.bfloat16)
    b2r = np.asarray(b2, f32).reshape(NCT, P)
    bprojr = np.asarray(bproj, f32).reshape(NCT, P)
    sp = np.arange(P)[:, None]
    qf = np.arange(512)[None, :]
    m01 = np.stack([(sp + 128 * off <= qf) for off in range(4)]).astype(
        ml_dtypes.bfloat16)
    ident = np.eye(P, dtype=f32)

    xT = [np.ascontiguousarray(x[b].T).reshape(NCT, P, T) for b in range(B)]

    def pair_weights(Wfull, g, scl):
        # raw pair weights [NPAIR, C, P]: cols 0:64 head g*8+2p, 64:128 head +1
        out = np.empty((NPAIR, C, P), f32)
        for p in range(NPAIR):
            hA, hB = g * 8 + 2 * p, g * 8 + 2 * p + 1
            out[p, :, 0:64] = Wfull[hA] * scl
            out[p, :, 64:128] = Wfull[hB] * scl
        return out

    per_g = {}
    for g in range(2):
        d = {}
        wqkv = np.empty((NPAIR, NCT, P, 3 * P), f32)
        ccqkv = np.empty((NPAIR, 2, 3 * P), f32)
        for j, (Wfull, scl) in enumerate(((Wq, scale), (Wk, 1.0), (Wv, 1.0))):
            raw = pair_weights(Wfull, g, scl)      # [NPAIR, C, P]
            wt = g1[None, :, None] * raw           # g1-folded
            wqkv[:, :, :, j * P:(j + 1) * P] = wt.reshape(NPAIR, NCT, P, P)
            ccqkv[:, 0, j * P:(j + 1) * P] = wt.sum(1)
            ccqkv[:, 1, j * P:(j + 1) * P] = np.einsum("c,pcd->pd", beta1, raw)
        d["wqkv"] = wqkv
        d["ccqkv"] = ccqkv
        d["wproj"] = np.ascontiguousarray(
            Wproj[g * 512:(g + 1) * 512]).reshape(NPAIR, P, C).astype(
                ml_dtypes.bfloat16)
        per_g[g] = d

    in_maps = []
    for c in range(8):
        b, g = c // 2, c % 2
        m = {
            "xT": xT[b],
            "xresT": np.ascontiguousarray(xT[b][:, :, g * TL:(g + 1) * TL]),
            "bproj": bprojr,
            "w1": w1bf,
            "ccf": ccfbf,
            "w2": w2bf,
            "b2": b2r,
            "m01": m01,
            "ident": ident,
        }
        m.update(per_g[g])
        in_maps.append(m)
    return in_maps


def kernel(**inputs):
    from concourse.bass_utils import run_bass_kernel_spmd

    if "nc" not in _CACHE:
        _CACHE["nc"] = _build(with_collective=True)
    nc = _CACHE["nc"]
    in_maps = _prep_inputs(**inputs)
    res = None
    last_err = None
    for _attempt in range(3):
        try:
            res = run_bass_kernel_spmd(nc, in_maps, list(range(8)))
            break
        except Exception as e:  # transient runtime/tunnel hiccups
            last_err = e
            import time
            time.sleep(10)
    if res is None:
        raise last_err
    out = np.empty((B, T, C), np.float32)
    for c in range(8):
        b, g = c // 2, c % 2
        outT = res.results[c]["outT"].reshape(C, TL)
        out[b, g * TL:(g + 1) * TL, :] = outT.T
    return out



# revision 6
# speedup vs baseline: 1.3680x; 1.3680x over previous
"""Trainium2 Bass kernel for a dense transformer block (pre-LN, causal MHA + FFN).

Sharding (8 NeuronCores): core c = 2*b + g handles sequence b (of B=4) and
half g (of 2): tensor-parallel attention over 8 of 16 heads (partial proj,
pairwise ReduceScatter over {2b, 2b+1}), then token-parallel LN2+FFN over
its 1024 of 2048 tokens.

Matmul strategy: fp8-e4m3 DoubleRow matmuls (0.5 cyc/row) everywhere except
the attention-score matmul (bf16). Accuracy-critical operands use 2-term fp8
splits (value + fp8 residual): Wproj, W1, W2 (host-side) and x2 (device-side,
Pool engine). LayerNorms are folded into the matmuls via augmented [1,2,*]
DoubleRow correction matmuls (colsum x (-mu), beta-dot x std) with per-column
rstd applied at eviction (Q, V) or inside exp (K, via activation scale).
Causal masking is done by accumulating ident20^T @ trilneg = -4800 onto the
diagonal stair blocks of S before exp (survives the ~1/32 exp scale).
"""
import numpy as np
import ml_dtypes
from contextlib import ExitStack

B, T, C = 4, 2048, 1024
H, HS = 16, 64
F = 4 * C
P = 128
EPS = 1e-5
NCT = C // P        # 8 c-tiles
NCP = NCT // 2      # 4 c-tile pairs
NPAIR = 4           # head-pairs per core
TL = T // 2         # 1024 local tokens
NFQ = 16            # f-tile pairs (FFN hidden 4096 = 32 tiles = 16 pairs)
GROUPS = [[0, 1], [2, 3], [4, 5], [6, 7]]
QC_ORDER = [0, 2, 1, 3]   # quarters 0,2 feed RS0; 1,3 feed RS1

E4 = ml_dtypes.float8_e4m3
BF = ml_dtypes.bfloat16

_CACHE = {}


def _build(with_collective=True):
    import concourse.tile as tile
    from concourse import bacc, mybir

    f32 = mybir.dt.float32
    f8 = mybir.dt.float8e4
    bf16 = mybir.dt.bfloat16
    AF = mybir.ActivationFunctionType
    OP = mybir.AluOpType
    PM = mybir.MatmulPerfMode

    nc = bacc.Bacc("TRN2", target_bir_lowering=False, debug=False, num_devices=8)

    # ---- DRAM I/O ----
    d_xa = nc.dram_tensor("xa", [NCP, P, 2, T], f8, kind="ExternalInput").ap()
    d_xsq = nc.dram_tensor("xsq", [NCP, P, 2, T], f8, kind="ExternalInput").ap()
    d_xres = nc.dram_tensor("xres", [NCT, P, TL], f32, kind="ExternalInput").ap()
    d_wqk = nc.dram_tensor("wqk", [NPAIR, P, NCP, 2, 256], f8,
                           kind="ExternalInput").ap()
    d_ccqk = nc.dram_tensor("ccqk", [NPAIR, 2, 256], f8, kind="ExternalInput").ap()
    d_wv = nc.dram_tensor("wv", [NPAIR, P, NCP, 2, P], f8,
                          kind="ExternalInput").ap()
    d_ccv = nc.dram_tensor("ccv", [NPAIR, 2, P], f8, kind="ExternalInput").ap()
    d_wpj = nc.dram_tensor("wpj", [2, 2, P, 2, C], f8, kind="ExternalInput").ap()
    d_w1 = nc.dram_tensor("w1", [2, NCP, P, 2, F], f8, kind="ExternalInput").ap()
    d_cc1 = nc.dram_tensor("cc1", [2, F], f8, kind="ExternalInput").ap()
    d_w2 = nc.dram_tensor("w2", [2, NCT, P, NFQ, 2, P], f8,
                          kind="ExternalInput").ap()
    d_b2 = nc.dram_tensor("b2", [NCT, P], f32, kind="ExternalInput").ap()
    d_tril = nc.dram_tensor("trilneg", [P, P], f8, kind="ExternalInput").ap()
    d_id20 = nc.dram_tensor("ident20", [P, P], f8, kind="ExternalInput").ap()
    d_out = nc.dram_tensor("outT", [NCT, P, TL], f32, kind="ExternalOutput").ap()

    with tile.TileContext(nc) as tc, ExitStack() as ctx:
        dram = ctx.enter_context(tc.tile_pool(name="dram", bufs=1, space="DRAM"))
        sa_bounce = [dram.tile([2, NCT, P, 512], bf16, name=f"sab{r}")
                     for r in range(2)]
        sa_local = [dram.tile([NCT, P, 512], bf16, name=f"sal{r}")
                    for r in range(2)]

        # ---- persistent constants / inputs ----
        const = ctx.enter_context(tc.tile_pool(name="const", bufs=1))
        tril = const.tile([P, P], f8)
        nc.sync.dma_start(tril[:], d_tril[:])
        id20 = const.tile([P, P], f8)
        nc.sync.dma_start(id20[:], d_id20[:])
        ones8 = const.tile([P, 2, 1], f8)
        nc.vector.memset(ones8[:], 1.0)
        ones11 = const.tile([1, 1], f32)
        nc.vector.memset(ones11[:], 1.0)
        ebias = const.tile([P, 1], f32)
        nc.vector.memset(ebias[:], -2.0)

        wat_pool = ctx.enter_context(tc.tile_pool(name="watt", bufs=1))
        wqk = [wat_pool.tile([P, NCP, 2, 256], f8, name=f"wqk{p}", tag=f"wqk{p}")
               for p in range(NPAIR)]
        ccqk = [wat_pool.tile([1, 2, 256], f8, name=f"ccqk{p}", tag=f"ccqk{p}")
                for p in range(NPAIR)]
        wv = [wat_pool.tile([P, NCP, 2, P], f8, name=f"wv{p}", tag=f"wv{p}")
              for p in range(NPAIR)]
        ccv = [wat_pool.tile([1, 2, P], f8, name=f"ccv{p}", tag=f"ccv{p}")
               for p in range(NPAIR)]
        wpj = [wat_pool.tile([P, 2, C], f8, name=f"wpj{s}{pp}", tag=f"wpj{s}{pp}")
               for s in range(2) for pp in range(2)]
        for p in range(NPAIR):
            nc.sync.dma_start(wqk[p][:], d_wqk[p])
            nc.sync.dma_start(ccqk[p][:], d_ccqk[p].unsqueeze(0))
            nc.sync.dma_start(wv[p][:], d_wv[p])
            nc.sync.dma_start(ccv[p][:], d_ccv[p].unsqueeze(0))
        for s in range(2):
            for pp in range(2):
                nc.sync.dma_start(wpj[s * 2 + pp][:], d_wpj[s, pp])

        # persistent row tiles (partition 0)
        rows_pool = ctx.enter_context(tc.tile_pool(name="rows", bufs=1))
        xaug = rows_pool.tile([1, 2, T], f8, tag="xaug")       # (-mu, std)
        xaug2 = rows_pool.tile([1, 2, TL], f8, tag="xaug2")

        bc_pool = ctx.enter_context(tc.tile_pool(name="bc", bufs=1))
        a1q = [bc_pool.tile([P, 512], f32, name=f"a1q{ch}", tag=f"a1q{ch}")
               for ch in range(4)]
        a2 = [bc_pool.tile([P, 512], f32, name=f"a2_{h}", tag=f"a2_{h}")
              for h in range(2)]
        rcol_pool = ctx.enter_context(tc.tile_pool(name="rcol", bufs=1))
        rvcol = rcol_pool.tile([P, 16], f32, tag="rvcol")      # rstd/32 by stripe
        # pin allocation order (pool frees must be LIFO vs first-use order)
        for t_ in (xaug, xaug2):
            nc.vector.memset(t_[0:1, 0:1], 0.0)
        for t_ in a1q + a2 + [rvcol]:
            nc.vector.memset(t_[:, 0:1], 0.0)

        x2_pool = ctx.enter_context(tc.tile_pool(name="x2f", bufs=1))
        x2sb = [x2_pool.tile([P, TL], f32, name=f"x2sb{co}", tag=f"x2sb{co}")
                for co in range(NCT)]
        x2q_pool = ctx.enter_context(tc.tile_pool(name="x2q", bufs=1))
        x2a = [x2q_pool.tile([P, 2, TL], f8, name=f"x2a{cp}", tag=f"x2a{cp}")
               for cp in range(NCP)]
        x2b = [x2q_pool.tile([P, 2, TL], f8, name=f"x2b{cp}", tag=f"x2b{cp}")
               for cp in range(NCP)]
        for t_ in x2sb + x2a + x2b:
            nc.vector.memset(t_[:, 0:1], 0.0)

        # attention working tiles (freed after proj)
        pattn = ExitStack()
        qk_pool = pattn.enter_context(tc.tile_pool(name="qk", bufs=1))
        qq = [qk_pool.tile([P, T], bf16, name=f"qq{p}", tag=f"qq{p}")
              for p in range(NPAIR)]
        kk = [qk_pool.tile([P, T], bf16, name=f"kk{p}", tag=f"kk{p}")
              for p in range(NPAIR)]
        for t_ in qq + kk:
            nc.vector.memset(t_[:, 0:1], 0.0)
        va_pool = pattn.enter_context(tc.tile_pool(name="va", bufs=1))
        v_aug = {}
        for p in range(NPAIR):
            for hh in range(2):
                for sp in range(8):
                    va = va_pool.tile([P, 2, 65], f8, name=f"va{p}_{hh}_{sp}",
                                      tag=f"va{p}_{hh}_{sp}")
                    nc.vector.memset(va[:, :, 64:65], 1.0)
                    v_aug[(p, hh, sp)] = va
        ediag_pool = pattn.enter_context(tc.tile_pool(name="ediag", bufs=1))
        e_diag = {}
        for p in range(NPAIR):
            for hh in range(2):
                for di in range(2):
                    et = ediag_pool.tile([P, 2, 512], f8,
                                         name=f"ed{p}_{hh}_{di}",
                                         tag=f"ed{p}_{hh}_{di}")
                    for j in range(2):
                        z = (2 * di + j) * P
                        if z:
                            nc.vector.memset(et[:, j, 0:z], 0.0)
                    e_diag[(p, hh, di)] = et
        attT_pool = pattn.enter_context(tc.tile_pool(name="attT", bufs=1))
        attT = [attT_pool.tile([P, 2, T], f8, name=f"attT{pp}", tag=f"attT{pp}")
                for pp in range(2)]
        for pp in range(2):
            nc.vector.memset(attT[pp][:, :, 0:1], 0.0)  # pin alloc order

        pxin = ExitStack()
        xin_pool = pxin.enter_context(tc.tile_pool(name="xin", bufs=1))
        xa = [xin_pool.tile([P, 2, T], f8, name=f"xa{cp}", tag=f"xa{cp}")
              for cp in range(NCP)]
        for cp in range(NCP):
            nc.sync.dma_start(xa[cp][:], d_xa[cp])
        pxsq = ExitStack()
        xsq_pool = pxsq.enter_context(tc.tile_pool(name="xsq", bufs=1))
        xsq = [xsq_pool.tile([P, 2, T], f8, name=f"xsq{cp}", tag=f"xsq{cp}")
               for cp in range(NCP)]
        for cp in range(NCP):
            nc.sync.dma_start(xsq[cp][:], d_xsq[cp])

        # =========== Phase 1: LN1 stats ===========
        p1 = ExitStack()
        st_ps1 = p1.enter_context(tc.tile_pool(name="stps1", bufs=2, space="PSUM"))
        row1_pool = p1.enter_context(tc.tile_pool(name="row1", bufs=6))
        for ch in range(4):
            sl = slice(ch * 512, (ch + 1) * 512)
            sx = st_ps1.tile([1, 512], f32, tag="sx")
            sq = st_ps1.tile([1, 512], f32, tag="sq")
            for cp in range(NCP):
                nc.tensor.matmul(sx[:], ones8[:], xa[cp][:, :, sl],
                                 start=(cp == 0), stop=(cp == NCP - 1),
                                 perf_mode=PM.DoubleRow)
                nc.tensor.matmul(sq[:], ones8[:], xsq[cp][:, :, sl],
                                 start=(cp == 0), stop=(cp == NCP - 1),
                                 perf_mode=PM.DoubleRow)
            # -mu (fp8 aug row) and f32 rows
            nc.scalar.activation(xaug[0:1, 0, sl], sx[:], AF.Copy, scale=-1.0 / C)
            mu = row1_pool.tile([1, 512], f32, tag="r")
            nc.scalar.activation(mu[:], sx[:], AF.Copy, scale=1.0 / C)
            ex2 = row1_pool.tile([1, 512], f32, tag="r")
            nc.scalar.activation(ex2[:], sq[:], AF.Copy, scale=1.0 / C)
            var = row1_pool.tile([1, 512], f32, tag="r")
            nc.vector.tensor_mul(var[:], mu[:], mu[:])
            nc.vector.scalar_tensor_tensor(var[:], ex2[:], EPS, var[:],
                                           OP.add, OP.subtract)
            std = row1_pool.tile([1, 512], f32, tag="r")
            nc.scalar.activation(std[:], var[:], AF.Sqrt)
            nc.vector.tensor_copy(xaug[0:1, 1, sl], std[:])
            rstd = row1_pool.tile([1, 512], f32, tag="r")
            nc.vector.reciprocal(rstd[:], std[:])
            r16 = row1_pool.tile([1, 512], f32, tag="r")
            nc.scalar.activation(r16[:], rstd[:], AF.Copy, scale=1.0 / 16)
            r32 = row1_pool.tile([1, 512], f32, tag="r")
            nc.scalar.activation(r32[:], rstd[:], AF.Copy, scale=1.0 / 32)
            nc.gpsimd.partition_broadcast(a1q[ch][:], r16[:])
            # rstd/32 per-stripe columns via mini PE transposes
            rc_ps = st_ps1.tile([P, 4], f32, tag="rc")
            for si in range(4):
                nc.tensor.transpose(rc_ps[:, si:si + 1],
                                    r32[:, si * P:(si + 1) * P], ones11[:])
            nc.vector.tensor_copy(rvcol[:, ch * 4:(ch + 1) * 4], rc_ps[:])
        p1.close()
        pxsq.close()

        # =========== Phase 2: QKV (all pairs) ===========
        p2 = ExitStack()
        qkps = p2.enter_context(tc.tile_pool(name="qkps", bufs=2, space="PSUM"))
        vps = p2.enter_context(tc.tile_pool(name="vps", bufs=4, space="PSUM"))
        for p in range(NPAIR):
            for ch in range(4):
                sl = slice(ch * 512, (ch + 1) * 512)
                q_ps = qkps.tile([P, 512], f32, tag="q")
                k_ps = qkps.tile([P, 512], f32, tag="k")
                for cp in range(NCP):
                    nc.tensor.matmul(q_ps[:], wqk[p][:, cp, :, 0:P],
                                     xa[cp][:, :, sl], start=(cp == 0),
                                     stop=False, perf_mode=PM.DoubleRow)
                    nc.tensor.matmul(k_ps[:], wqk[p][:, cp, :, P:256],
                                     xa[cp][:, :, sl], start=(cp == 0),
                                     stop=False, perf_mode=PM.DoubleRow)
                nc.tensor.matmul(q_ps[:], ccqk[p][:, :, 0:P], xaug[:, :, sl],
                                 start=False, stop=True, perf_mode=PM.DoubleRow)
                nc.tensor.matmul(k_ps[:], ccqk[p][:, :, P:256], xaug[:, :, sl],
                                 start=False, stop=True, perf_mode=PM.DoubleRow)
                nc.vector.tensor_mul(qq[p][:, sl], q_ps[:], a1q[ch][:])
                nc.scalar.copy(kk[p][:, sl], k_ps[:])
            for st in range(16):
                ssl = slice(st * P, (st + 1) * P)
                v_ps = vps.tile([P, P], f32, tag="v")
                for cp in range(NCP):
                    nc.tensor.matmul(v_ps[:], xa[cp][:, :, ssl], wv[p][:, cp],
                                     start=(cp == 0), stop=False,
                                     perf_mode=PM.DoubleRow)
                nc.tensor.matmul(v_ps[:], xaug[:, :, ssl], ccv[p][:],
                                 start=False, stop=True, perf_mode=PM.DoubleRow)
                for hh in range(2):
                    nc.vector.tensor_scalar_mul(
                        v_aug[(p, hh, st // 2)][:, st % 2, 0:64],
                        v_ps[:, hh * 64:(hh + 1) * 64], rvcol[:, st:st + 1])
        p2.close()
        pxin.close()

        # ===== Phase 3+4: attention (qc-major) + proj + ReduceScatter =====
        p3 = ExitStack()
        st_ps = p3.enter_context(tc.tile_pool(name="stps", bufs=4, space="PSUM"))
        att_ps = p3.enter_context(tc.tile_pool(name="attps", bufs=2, space="PSUM"))
        pj_ps = p3.enter_context(tc.tile_pool(name="pjps", bufs=2, space="PSUM"))
        sasb_pool = p3.enter_context(tc.tile_pool(name="sasb", bufs=4))
        end_pool = p3.enter_context(tc.tile_pool(name="endp", bufs=4))
        rec_pool = p3.enter_context(tc.tile_pool(name="recp", bufs=2))
        bcr_pool = p3.enter_context(tc.tile_pool(name="bcrp", bufs=2))

        for qc in QC_ORDER:
            qsl = slice(qc * 512, (qc + 1) * 512)
            n_sp = 2 * (qc + 1)
            for p in range(NPAIR):
                for hh in range(2):
                    hsl = slice(hh * 64, (hh + 1) * 64)
                    att = att_ps.tile([65, 512], f32, tag="att")
                    for spi in range(n_sp):
                        diag = (spi >= 2 * qc)
                        if diag:
                            et = e_diag[(p, hh, spi - 2 * qc)]
                        else:
                            et = end_pool.tile([P, 2, 512], f8, tag="e")
                        for j in range(2):
                            si = 2 * spi + j
                            ssl = slice(si * P, (si + 1) * P)
                            stp = st_ps.tile([P, 512], f32, tag="st")
                            off = si - 4 * qc
                            if diag:
                                nc.tensor.matmul(stp[:], kk[p][hsl, ssl],
                                                 qq[p][hsl, qsl],
                                                 start=True, stop=False)
                                nc.tensor.matmul(stp[:, off * P:(off + 1) * P],
                                                 id20[:], tril[:],
                                                 start=False, stop=True,
                                                 skip_group_check=True)
                                nc.scalar.activation(
                                    et[0:P, j, off * P:512],
                                    stp[:, off * P:512], AF.Exp, bias=ebias[:],
                                    scale=rvcol[:, si:si + 1])
                            else:
                                nc.tensor.matmul(stp[:], kk[p][hsl, ssl],
                                                 qq[p][hsl, qsl],
                                                 start=True, stop=True)
                                nc.scalar.activation(
                                    et[0:P, j, :], stp[:], AF.Exp,
                                    bias=ebias[:], scale=rvcol[:, si:si + 1])
                        nc.tensor.matmul(att[:], v_aug[(p, hh, spi)][:], et[:],
                                         start=(spi == 0), stop=(spi == n_sp - 1),
                                         perf_mode=PM.DoubleRow)
                    rec = rec_pool.tile([1, 512], f32, tag="rec")
                    nc.vector.reciprocal(rec[:], att[64:65, :])
                    bcr = bcr_pool.tile([64, 512], f32, tag="bcr")
                    nc.gpsimd.partition_broadcast(bcr[:], rec[:])
                    nc.vector.tensor_mul(attT[p // 2][hsl, p % 2, qsl],
                                         att[0:64, :], bcr[:])
            # proj for this quarter (both wproj split terms)
            r, fold = qc % 2, qc // 2
            for co in range(NCT):
                pp_ps = pj_ps.tile([P, 512], f32, tag="pp")
                for pp in range(2):
                    for s in range(2):
                        nc.tensor.matmul(pp_ps[:],
                                         wpj[s * 2 + pp][:, :, co * P:(co + 1) * P],
                                         attT[pp][:, :, qsl],
                                         start=(pp == 0 and s == 0),
                                         stop=(pp == 1 and s == 1),
                                         perf_mode=PM.DoubleRow)
                sasb = sasb_pool.tile([P, 512], bf16, tag="sasb")
                nc.scalar.copy(sasb[:], pp_ps[:])
                nc.sync.dma_start(sa_bounce[r][fold, co], sasb[:])
            if fold == 1:
                if with_collective:
                    nc.gpsimd.collective_compute(
                        "ReduceScatter", OP.add, replica_groups=GROUPS,
                        ins=[sa_bounce[r].opt()], outs=[sa_local[r].opt()])
                else:
                    nc.sync.dma_start(sa_local[r][:], sa_bounce[r][0])
        p3.close()
        pattn.close()

        # =========== Phase 5: x2 build (Pool) + LN2 stats ===========
        px2 = ExitStack()
        x2sq_pool = px2.enter_context(tc.tile_pool(name="x2sq", bufs=1))
        x2sq = [x2sq_pool.tile([P, 2, TL], f8, name=f"x2sq{cp}", tag=f"x2sq{cp}")
                for cp in range(NCP)]
        xres_pool = px2.enter_context(tc.tile_pool(name="xres", bufs=4))

        for r in range(2):
            lsl = slice(r * 512, (r + 1) * 512)
            for co in range(NCT):
                cp, j = co // 2, co % 2
                sal = xres_pool.tile([P, 512], bf16, tag="sal")
                nc.sync.dma_start(sal[:], sa_local[r][co])
                xre = xres_pool.tile([P, 512], f32, tag="xre")
                nc.sync.dma_start(xre[:], d_xres[co][:, lsl])
                nc.gpsimd.scalar_tensor_tensor(x2sb[co][:, lsl], sal[:],
                                               1.0 / 32, xre[:],
                                               OP.mult, OP.add)
                nc.gpsimd.tensor_copy(x2a[cp][:, j, lsl], x2sb[co][:, lsl])
                nc.gpsimd.scalar_tensor_tensor(x2b[cp][:, j, lsl],
                                               x2sb[co][:, lsl], 0.0,
                                               x2a[cp][:, j, lsl],
                                               OP.add, OP.subtract)
                nc.gpsimd.tensor_mul(x2sq[cp][:, j, lsl], x2sb[co][:, lsl],
                                     x2sb[co][:, lsl])

        p5 = ExitStack()
        st_ps2 = p5.enter_context(tc.tile_pool(name="stps2", bufs=2, space="PSUM"))
        row2_pool = p5.enter_context(tc.tile_pool(name="row2", bufs=4))
        for h in range(2):
            lsl = slice(h * 512, (h + 1) * 512)
            sx = st_ps2.tile([1, 512], f32, tag="sx2")
            sq = st_ps2.tile([1, 512], f32, tag="sq2")
            for cp in range(NCP):
                nc.tensor.matmul(sx[:], ones8[:], x2a[cp][:, :, lsl],
                                 start=(cp == 0), stop=False,
                                 perf_mode=PM.DoubleRow)
                nc.tensor.matmul(sq[:], ones8[:], x2sq[cp][:, :, lsl],
                                 start=(cp == 0), stop=(cp == NCP - 1),
                                 perf_mode=PM.DoubleRow)
            for cp in range(NCP):
                nc.tensor.matmul(sx[:], ones8[:], x2b[cp][:, :, lsl],
                                 start=False, stop=(cp == NCP - 1),
                                 perf_mode=PM.DoubleRow)
            nc.scalar.activation(xaug2[0:1, 0, lsl], sx[:], AF.Copy,
                                 scale=-1.0 / C)
            mu = row2_pool.tile([1, 512], f32, tag="r")
            nc.scalar.activation(mu[:], sx[:], AF.Copy, scale=1.0 / C)
            ex2 = row2_pool.tile([1, 512], f32, tag="r")
            nc.scalar.activation(ex2[:], sq[:], AF.Copy, scale=1.0 / C)
            var = row2_pool.tile([1, 512], f32, tag="r")
            nc.vector.tensor_mul(var[:], mu[:], mu[:])
            nc.vector.scalar_tensor_tensor(var[:], ex2[:], EPS, var[:],
                                           OP.add, OP.subtract)
            std = row2_pool.tile([1, 512], f32, tag="r")
            nc.scalar.activation(std[:], var[:], AF.Sqrt)
            nc.vector.tensor_copy(xaug2[0:1, 1, lsl], std[:])
            rs2 = row2_pool.tile([1, 512], f32, tag="r")
            nc.vector.reciprocal(rs2[:], std[:])
            rs2s = row2_pool.tile([1, 512], f32, tag="r")
            nc.scalar.activation(rs2s[:], rs2[:], AF.Copy, scale=1.0 / 1024)
            nc.gpsimd.partition_broadcast(a2[h][:], rs2s[:])
        p5.close()
        px2.close()

        # =========== Phase 6: FFN ===========
        p6 = ExitStack()
        w1_pool = p6.enter_context(tc.tile_pool(name="w1", bufs=2))
        cc1_pool = p6.enter_context(tc.tile_pool(name="cc1", bufs=2))
        relu_pool = p6.enter_context(tc.tile_pool(name="relu", bufs=1))
        ffn_ps = p6.enter_context(tc.tile_pool(name="ffnps", bufs=4, space="PSUM"))
        relu = [relu_pool.tile([P, 2, TL], f8, name=f"rl{fq}", tag=f"rl{fq}")
                for fq in range(NFQ)]

        for fog in range(8):
            gsl = slice(fog * 512, (fog + 1) * 512)
            w1t = []
            for sp_ in range(2):
                for cp in range(NCP):
                    w = w1_pool.tile([P, 2, 512], f8, tag=f"w1_{sp_}{cp}")
                    nc.sync.dma_start(w[:], d_w1[sp_, cp][:, :, gsl])
                    w1t.append(w)
            cc1 = cc1_pool.tile([1, 2, 512], f8, tag="cc1")
            nc.sync.dma_start(cc1[:], d_cc1[:, gsl].unsqueeze(0))
            for fol in range(4):
                fo = fog * 4 + fol
                fsl = slice(fol * P, (fol + 1) * P)
                fq, fj = fo // 2, fo % 2
                for h in range(2):
                    lsl = slice(h * 512, (h + 1) * 512)
                    fp = ffn_ps.tile([P, 512], f32, tag="fp")
                    for cp in range(NCP):
                        nc.tensor.matmul(fp[:], w1t[cp][:, :, fsl],
                                         x2a[cp][:, :, lsl], start=(cp == 0),
                                         stop=False, perf_mode=PM.DoubleRow)
                    for cp in range(NCP):
                        nc.tensor.matmul(fp[:], w1t[NCP + cp][:, :, fsl],
                                         x2a[cp][:, :, lsl], start=False,
                                         stop=False, perf_mode=PM.DoubleRow)
                    for cp in range(NCP):
                        nc.tensor.matmul(fp[:], w1t[cp][:, :, fsl],
                                         x2b[cp][:, :, lsl], start=False,
                                         stop=False, perf_mode=PM.DoubleRow)
                    nc.tensor.matmul(fp[:], cc1[:, :, fsl], xaug2[:, :, lsl],
                                     start=False, stop=True,
                                     perf_mode=PM.DoubleRow)
                    if fo % 2 == 0:
                        nc.scalar.activation(relu[fq][:, fj, lsl], fp[:], AF.Relu)
                    else:
                        nc.vector.tensor_scalar_max(relu[fq][:, fj, lsl], fp[:],
                                                    0.0)

        w2_pool = p6.enter_context(tc.tile_pool(name="w2", bufs=4))
        out_pool = p6.enter_context(tc.tile_pool(name="outsb", bufs=4))
        b2_pool = p6.enter_context(tc.tile_pool(name="b2p", bufs=1))
        b2col = [b2_pool.tile([P, 1], f32, name=f"b2c{co}", tag=f"b2c{co}")
                 for co in range(NCT)]
        for co in range(NCT):
            nc.sync.dma_start(b2col[co][:], d_b2[co].unsqueeze(1))
        for co in range(NCT):
            w2a = w2_pool.tile([P, NFQ, 2, P], f8, tag="w2t")
            nc.sync.dma_start(w2a[:], d_w2[0, co])
            w2b = w2_pool.tile([P, NFQ, 2, P], f8, tag="w2t")
            nc.sync.dma_start(w2b[:], d_w2[1, co])
            for h in range(2):
                lsl = slice(h * 512, (h + 1) * 512)
                fp = ffn_ps.tile([P, 512], f32, tag="fp2")
                for fq in range(NFQ):
                    nc.tensor.matmul(fp[:], w2a[:, fq], relu[fq][:, :, lsl],
                                     start=(fq == 0), stop=False,
                                     perf_mode=PM.DoubleRow)
                for fq in range(NFQ):
                    nc.tensor.matmul(fp[:], w2b[:, fq], relu[fq][:, :, lsl],
                                     start=False, stop=(fq == NFQ - 1),
                                     perf_mode=PM.DoubleRow)
                tmp = out_pool.tile([P, 512], f32, tag="tmp")
                nc.vector.tensor_mul(tmp[:], fp[:], a2[h][:])
                osb = out_pool.tile([P, 512], f32, tag="osb")
                nc.vector.scalar_tensor_tensor(osb[:], tmp[:], b2col[co][:],
                                               x2sb[co][:, lsl], OP.add, OP.add)
                nc.sync.dma_start(d_out[co][:, lsl], osb[:])
        p6.close()

    nc.compile()
    return nc


def _q8(v):
    return np.asarray(v, np.float32).astype(E4)


def _prep_inputs(x, Wq, Wk, Wv, Wproj, bproj, W1, b1, W2, b2, g1, beta1, g2,
                 beta2):
    f32 = np.float32
    scale = HS ** -0.5
    x = np.asarray(x, f32)
    Wq = np.asarray(Wq, f32); Wk = np.asarray(Wk, f32); Wv = np.asarray(Wv, f32)
    Wproj = np.asarray(Wproj, f32); bproj = np.asarray(bproj, f32)
    W1 = np.asarray(W1, f32); b1 = np.asarray(b1, f32)
    W2 = np.asarray(W2, f32); b2 = np.asarray(b2, f32)
    g1 = np.asarray(g1, f32); beta1 = np.asarray(beta1, f32)
    g2 = np.asarray(g2, f32); beta2 = np.asarray(beta2, f32)

    # ---- shared (g-independent) weights ----
    w1s = (g2[:, None] * W1) * 32.0                       # [C, F]
    w1a = _q8(w1s)
    w1b = _q8(w1s - w1a.astype(f32))
    w1_pack = np.stack([w1a.reshape(NCP, 2, P, F).transpose(0, 2, 1, 3),
                        w1b.reshape(NCP, 2, P, F).transpose(0, 2, 1, 3)])
    cc1 = np.empty((2, F), f32)
    cc1[0] = (w1a.astype(f32) + w1b.astype(f32)).sum(0)
    cc1[1] = (b1 + beta2 @ W1) * 32.0
    cc1 = _q8(cc1)

    w2s = W2 * 32.0                                        # [F, C]
    w2a = _q8(w2s)
    w2b = _q8(w2s - w2a.astype(f32))
    # [2, NCT, P, NFQ, 2, P]: w2[s, co, p, fq, j, cc] = w2s[(2fq+j)*128+p, co*128+cc]
    def pack_w2(w):
        return np.ascontiguousarray(
            w.reshape(NFQ, 2, P, NCT, P).transpose(3, 2, 0, 1, 4))
    w2_pack = np.stack([pack_w2(w2a), pack_w2(w2b)])

    b2r = b2.reshape(NCT, P)
    kp = np.arange(P)[:, None]
    lq = np.arange(P)[None, :]
    trilneg = (-240.0 * (lq < kp)).astype(E4)
    ident20 = (20.0 * np.eye(P)).astype(E4)

    # ---- per-batch x ----
    xa_b, xsq_b, xresT_b = [], [], []
    for b in range(B):
        xT = np.ascontiguousarray(x[b].T)                  # [C, T]
        xq = _q8(xT)
        xa_b.append(np.ascontiguousarray(
            xq.reshape(NCP, 2, P, T).transpose(0, 2, 1, 3)))
        xsq_b.append(np.ascontiguousarray(
            _q8(xq.astype(f32) ** 2).reshape(NCP, 2, P, T).transpose(0, 2, 1, 3)))
        xresT_b.append(xT + bproj[:, None])                # bproj folded in

    # ---- per-group attention weights ----
    per_g = {}
    for g in range(2):
        wqk = np.empty((NPAIR, P, NCP, 2, 256), E4)
        ccqk = np.empty((NPAIR, 2, 256), f32)
        wv_ = np.empty((NPAIR, P, NCP, 2, P), E4)
        ccv = np.empty((NPAIR, 2, P), f32)
        for p in range(NPAIR):
            hA, hB = g * 8 + 2 * p, g * 8 + 2 * p + 1
            for (Wfull, scl, col) in ((Wq, scale * 16.0, slice(0, P)),
                                      (Wk, 32.0, slice(P, 256))):
                wt = np.concatenate([Wfull[hA], Wfull[hB]], axis=1) * scl  # [C,128]
                wq8 = _q8(g1[:, None] * wt)
                wqk[p, :, :, :, col] = wq8.reshape(NCP, 2, P, P).transpose(
                    2, 0, 1, 3)
                ccqk[p, 0, col] = wq8.astype(f32).sum(0)
                ccqk[p, 1, col] = beta1 @ wt
            wt = np.concatenate([Wv[hA], Wv[hB]], axis=1) * 32.0
            wq8 = _q8(g1[:, None] * wt)
            wv_[p] = wq8.reshape(NCP, 2, P, P).transpose(2, 0, 1, 3)
            ccv[p, 0] = wq8.astype(f32).sum(0)
            ccv[p, 1] = beta1 @ wt
        # wpj [2(split), 2(pp), P, 2(j), C]: wps[pp, j, f, c] -> [pp, f, j, c]
        wps = np.ascontiguousarray(
            Wproj[g * 512:(g + 1) * 512] * 32.0).reshape(2, 2, P, C)
        wpa = _q8(wps)
        wpb = _q8(wps - wpa.astype(f32))
        wpj = np.stack([wpa.transpose(0, 2, 1, 3), wpb.transpose(0, 2, 1, 3)])
        per_g[g] = dict(wqk=wqk, ccqk=_q8(ccqk), wv=wv_, ccv=_q8(ccv),
                        wpj=np.ascontiguousarray(wpj))

    in_maps = []
    for c in range(8):
        b, g = c // 2, c % 2
        m = dict(
            xa=xa_b[b], xsq=xsq_b[b],
            xres=np.ascontiguousarray(
                xresT_b[b][:, g * TL:(g + 1) * TL].reshape(NCT, P, TL)),
            w1=w1_pack, cc1=cc1, w2=w2_pack, b2=b2r,
            trilneg=trilneg, ident20=ident20,
        )
        m.update(per_g[g])
        in_maps.append(m)
    return in_maps


def kernel(**inputs):
    from concourse.bass_utils import run_bass_kernel_spmd

    if "nc" not in _CACHE:
        _CACHE["nc"] = _build(with_collective=True)
    nc = _CACHE["nc"]
    in_maps = _prep_inputs(**inputs)
    res = None
    last_err = None
    for _attempt in range(3):
        try:
            res = run_bass_kernel_spmd(nc, in_maps, list(range(8)))
            break
        except Exception as e:  # transient runtime/tunnel hiccups
            last_err = e
            import time
            time.sleep(10)
    if res is None:
        raise last_err
    out = np.empty((B, T, C), np.float32)
    for c in range(8):
        b, g = c // 2, c % 2
        outT = res.results[c]["outT"].reshape(C, TL)
        out[b, g * TL:(g + 1) * TL, :] = outT.T
    return out


# revision 8
# speedup vs baseline: 1.3980x; 1.0220x over previous
"""Trainium2 Bass kernel for a dense transformer block (pre-LN, causal MHA + FFN).

Sharding (8 NeuronCores): core c = 2*b + g handles sequence b (of B=4) and
half g (of 2): tensor-parallel attention over 8 of 16 heads (partial proj,
pairwise ReduceScatter over {2b, 2b+1}), then token-parallel LN2+FFN over
its 1024 of 2048 tokens.

Matmul strategy: fp8-e4m3 DoubleRow matmuls (0.5 cyc/row) everywhere except
the attention-score matmul (bf16). Accuracy-critical operands use 2-term fp8
splits (value + fp8 residual): Wproj, W1, W2 (host-side) and x2 (device-side,
Pool engine). LayerNorms are folded into the matmuls via augmented [1,2,*]
DoubleRow correction matmuls (colsum x (-mu), beta-dot x std) with per-column
rstd applied at eviction (Q, V) or inside exp (K, via activation scale).
Causal masking is done by accumulating ident20^T @ trilneg = -4800 onto the
diagonal stair blocks of S before exp (survives the ~1/32 exp scale).
"""
import numpy as np
import ml_dtypes
from contextlib import ExitStack

B, T, C = 4, 2048, 1024
H, HS = 16, 64
F = 4 * C
P = 128
EPS = 1e-5
NCT = C // P        # 8 c-tiles
NCP = NCT // 2      # 4 c-tile pairs
NPAIR = 4           # head-pairs per core
TL = T // 2         # 1024 local tokens
NFQ = 16            # f-tile pairs (FFN hidden 4096 = 32 tiles = 16 pairs)
GROUPS = [[0, 1], [2, 3], [4, 5], [6, 7]]
QC_ORDER = [0, 2, 1, 3]   # quarters 0,2 feed RS0; 1,3 feed RS1

E4 = ml_dtypes.float8_e4m3
BF = ml_dtypes.bfloat16

_CACHE = {}


def _build(with_collective=True):
    import concourse.tile as tile
    from concourse import bacc, mybir

    f32 = mybir.dt.float32
    f8 = mybir.dt.float8e4
    bf16 = mybir.dt.bfloat16
    AF = mybir.ActivationFunctionType
    OP = mybir.AluOpType
    PM = mybir.MatmulPerfMode

    nc = bacc.Bacc("TRN2", target_bir_lowering=False, debug=False, num_devices=8)

    # ---- DRAM I/O ----
    d_xa = nc.dram_tensor("xa", [NCP, P, 2, T], f8, kind="ExternalInput").ap()
    d_xsq = nc.dram_tensor("xsq", [NCP, P, 2, T], f8, kind="ExternalInput").ap()
    d_xres = nc.dram_tensor("xres", [NCT, P, TL], f32, kind="ExternalInput").ap()
    d_wqk = nc.dram_tensor("wqk", [NPAIR, P, NCP, 2, 256], f8,
                           kind="ExternalInput").ap()
    d_ccqk = nc.dram_tensor("ccqk", [NPAIR, 2, 256], f8, kind="ExternalInput").ap()
    d_wv = nc.dram_tensor("wv", [NPAIR, P, NCP, 2, P], f8,
                          kind="ExternalInput").ap()
    d_ccv = nc.dram_tensor("ccv", [NPAIR, 2, P], f8, kind="ExternalInput").ap()
    d_wpj = nc.dram_tensor("wpj", [2, 2, P, 2, C], f8, kind="ExternalInput").ap()
    d_w1 = nc.dram_tensor("w1", [2, NCP, P, 2, F], f8, kind="ExternalInput").ap()
    d_cc1 = nc.dram_tensor("cc1", [2, F], f8, kind="ExternalInput").ap()
    d_w2 = nc.dram_tensor("w2", [2, NCT, P, NFQ, 2, P], f8,
                          kind="ExternalInput").ap()
    d_b2 = nc.dram_tensor("b2", [NCT, P], f32, kind="ExternalInput").ap()
    d_tril = nc.dram_tensor("trilneg", [P, P], f8, kind="ExternalInput").ap()
    d_id20 = nc.dram_tensor("ident20", [P, P], f8, kind="ExternalInput").ap()
    d_out = nc.dram_tensor("outT", [NCT, P, TL], f32, kind="ExternalOutput").ap()

    with tile.TileContext(nc) as tc, ExitStack() as ctx:
        dram = ctx.enter_context(tc.tile_pool(name="dram", bufs=1, space="DRAM"))
        sa_bounce = [dram.tile([2, NCT, P, 512], bf16, name=f"sab{r}")
                     for r in range(2)]
        sa_local = [dram.tile([NCT, P, 512], bf16, name=f"sal{r}")
                    for r in range(2)]

        # ---- persistent constants / inputs ----
        const = ctx.enter_context(tc.tile_pool(name="const", bufs=1))
        tril = const.tile([P, P], f8)
        nc.sync.dma_start(tril[:], d_tril[:])
        id20 = const.tile([P, P], f8)
        nc.sync.dma_start(id20[:], d_id20[:])
        ones8 = const.tile([P, 2, 1], f8)
        nc.vector.memset(ones8[:], 1.0)
        ones11 = const.tile([1, 1], f32)
        nc.vector.memset(ones11[:], 1.0)
        ebias = const.tile([P, 1], f32)
        nc.vector.memset(ebias[:], -2.0)

        wat_pool = ctx.enter_context(tc.tile_pool(name="watt", bufs=1))
        wqk = [wat_pool.tile([P, NCP, 2, 256], f8, name=f"wqk{p}", tag=f"wqk{p}")
               for p in range(NPAIR)]
        ccqk = [wat_pool.tile([1, 2, 256], f8, name=f"ccqk{p}", tag=f"ccqk{p}")
                for p in range(NPAIR)]
        wv = [wat_pool.tile([P, NCP, 2, P], f8, name=f"wv{p}", tag=f"wv{p}")
              for p in range(NPAIR)]
        ccv = [wat_pool.tile([1, 2, P], f8, name=f"ccv{p}", tag=f"ccv{p}")
               for p in range(NPAIR)]
        wpj = [wat_pool.tile([P, 2, C], f8, name=f"wpj{s}{pp}", tag=f"wpj{s}{pp}")
               for s in range(2) for pp in range(2)]
        for p in range(NPAIR):
            nc.sync.dma_start(wqk[p][:], d_wqk[p])
            nc.sync.dma_start(ccqk[p][:], d_ccqk[p].unsqueeze(0))
            nc.sync.dma_start(wv[p][:], d_wv[p])
            nc.sync.dma_start(ccv[p][:], d_ccv[p].unsqueeze(0))
        for s in range(2):
            for pp in range(2):
                nc.sync.dma_start(wpj[s * 2 + pp][:], d_wpj[s, pp])

        # persistent row tiles (partition 0)
        rows_pool = ctx.enter_context(tc.tile_pool(name="rows", bufs=1))
        xaug = rows_pool.tile([1, 2, T], f8, tag="xaug")       # (-mu, std)
        xaug2 = rows_pool.tile([1, 2, TL], f8, tag="xaug2")

        bc_pool = ctx.enter_context(tc.tile_pool(name="bc", bufs=1))
        a1q = [bc_pool.tile([P, 512], f32, name=f"a1q{ch}", tag=f"a1q{ch}")
               for ch in range(4)]
        a2 = [bc_pool.tile([P, 512], f32, name=f"a2_{h}", tag=f"a2_{h}")
              for h in range(2)]
        rcol_pool = ctx.enter_context(tc.tile_pool(name="rcol", bufs=1))
        rvcol = rcol_pool.tile([P, 16], f32, tag="rvcol")      # rstd/32 by stripe
        # pin allocation order (pool frees must be LIFO vs first-use order)
        for t_ in (xaug, xaug2):
            nc.gpsimd.memset(t_[0:1, 0:1], 0.0)
        for t_ in a1q + a2 + [rvcol]:
            nc.gpsimd.memset(t_[:, 0:1], 0.0)

        x2_pool = ctx.enter_context(tc.tile_pool(name="x2f", bufs=1))
        x2sb = [x2_pool.tile([P, TL], f32, name=f"x2sb{co}", tag=f"x2sb{co}")
                for co in range(NCT)]
        x2q_pool = ctx.enter_context(tc.tile_pool(name="x2q", bufs=1))
        x2a = [x2q_pool.tile([P, 2, TL], f8, name=f"x2a{cp}", tag=f"x2a{cp}")
               for cp in range(NCP)]
        x2b = [x2q_pool.tile([P, 2, TL], f8, name=f"x2b{cp}", tag=f"x2b{cp}")
               for cp in range(NCP)]
        for t_ in x2sb + x2a + x2b:
            nc.gpsimd.memset(t_[:, 0:1], 0.0)

        # attention working tiles (freed after proj)
        pattn = ExitStack()
        qk_pool = pattn.enter_context(tc.tile_pool(name="qk", bufs=1))
        qq = [qk_pool.tile([P, T], bf16, name=f"qq{p}", tag=f"qq{p}")
              for p in range(NPAIR)]
        kk = [qk_pool.tile([P, T], bf16, name=f"kk{p}", tag=f"kk{p}")
              for p in range(NPAIR)]
        for t_ in qq + kk:
            nc.gpsimd.memset(t_[:, 0:1], 0.0)
        va_pool = pattn.enter_context(tc.tile_pool(name="va", bufs=1))
        v_aug = {}
        for p in range(NPAIR):
            for sp in range(8):
                va = va_pool.tile([P, 2, 2, 65], f8, name=f"va{p}_{sp}",
                                  tag=f"va{p}_{sp}")
                nc.gpsimd.memset(va[:, :, :, 64:65], 1.0)
                v_aug[(p, sp)] = va
        ediag_pool = pattn.enter_context(tc.tile_pool(name="ediag", bufs=1))
        e_diag = {}
        for p in range(NPAIR):
            for hh in range(2):
                for di in range(2):
                    et = ediag_pool.tile([P, 2, 512], f8,
                                         name=f"ed{p}_{hh}_{di}",
                                         tag=f"ed{p}_{hh}_{di}")
                    for j in range(2):
                        z = (2 * di + j) * P
                        if z:
                            nc.gpsimd.memset(et[:, j, 0:z], 0.0)
                    e_diag[(p, hh, di)] = et
        attT_pool = pattn.enter_context(tc.tile_pool(name="attT", bufs=1))
        attT = [attT_pool.tile([P, 2, T], f8, name=f"attT{pp}", tag=f"attT{pp}")
                for pp in range(2)]
        for pp in range(2):
            nc.gpsimd.memset(attT[pp][:, :, 0:1], 0.0)  # pin alloc order

        pxin = ExitStack()
        xin_pool = pxin.enter_context(tc.tile_pool(name="xin", bufs=1))
        xa = [xin_pool.tile([P, 2, T], f8, name=f"xa{cp}", tag=f"xa{cp}")
              for cp in range(NCP)]
        for cp in range(NCP):
            nc.sync.dma_start(xa[cp][:], d_xa[cp])
        pxsq = ExitStack()
        xsq_pool = pxsq.enter_context(tc.tile_pool(name="xsq", bufs=1))
        xsq = [xsq_pool.tile([P, 2, T], f8, name=f"xsq{cp}", tag=f"xsq{cp}")
               for cp in range(NCP)]
        for cp in range(NCP):
            nc.sync.dma_start(xsq[cp][:], d_xsq[cp])

        # =========== Phase 1: LN1 stats ===========
        p1 = ExitStack()
        st_ps1 = p1.enter_context(tc.tile_pool(name="stps1", bufs=2, space="PSUM"))
        row1_pool = p1.enter_context(tc.tile_pool(name="row1", bufs=6))
        for ch in range(4):
            sl = slice(ch * 512, (ch + 1) * 512)
            sx = st_ps1.tile([1, 512], f32, tag="sx")
            sq = st_ps1.tile([1, 512], f32, tag="sq")
            for cp in range(NCP):
                nc.tensor.matmul(sx[:], ones8[:], xa[cp][:, :, sl],
                                 start=(cp == 0), stop=(cp == NCP - 1),
                                 perf_mode=PM.DoubleRow)
                nc.tensor.matmul(sq[:], ones8[:], xsq[cp][:, :, sl],
                                 start=(cp == 0), stop=(cp == NCP - 1),
                                 perf_mode=PM.DoubleRow)
            # -mu (fp8 aug row) and f32 rows
            nc.scalar.activation(xaug[0:1, 0, sl], sx[:], AF.Copy, scale=-1.0 / C)
            mu = row1_pool.tile([1, 512], f32, tag="r")
            nc.scalar.activation(mu[:], sx[:], AF.Copy, scale=1.0 / C)
            ex2 = row1_pool.tile([1, 512], f32, tag="r")
            nc.scalar.activation(ex2[:], sq[:], AF.Copy, scale=1.0 / C)
            var = row1_pool.tile([1, 512], f32, tag="r")
            nc.vector.tensor_mul(var[:], mu[:], mu[:])
            nc.vector.scalar_tensor_tensor(var[:], ex2[:], EPS, var[:],
                                           OP.add, OP.subtract)
            std = row1_pool.tile([1, 512], f32, tag="r")
            nc.scalar.activation(std[:], var[:], AF.Sqrt)
            nc.vector.tensor_copy(xaug[0:1, 1, sl], std[:])
            rstd = row1_pool.tile([1, 512], f32, tag="r")
            nc.vector.reciprocal(rstd[:], std[:])
            r16 = row1_pool.tile([1, 512], f32, tag="r")
            nc.scalar.activation(r16[:], rstd[:], AF.Copy, scale=1.0 / 16)
            r32 = row1_pool.tile([1, 512], f32, tag="r")
            nc.scalar.activation(r32[:], rstd[:], AF.Copy, scale=1.0 / 32)
            nc.gpsimd.partition_broadcast(a1q[ch][:], r16[:])
            # rstd/32 per-stripe columns via mini PE transposes
            rc_ps = st_ps1.tile([P, 4], f32, tag="rc")
            for si in range(4):
                nc.tensor.transpose(rc_ps[:, si:si + 1],
                                    r32[:, si * P:(si + 1) * P], ones11[:])
            nc.vector.tensor_copy(rvcol[:, ch * 4:(ch + 1) * 4], rc_ps[:])
        p1.close()
        pxsq.close()

        # =========== Phase 2: QKV (all pairs) ===========
        p2 = ExitStack()
        qkps = p2.enter_context(tc.tile_pool(name="qkps", bufs=2, space="PSUM"))
        vps = p2.enter_context(tc.tile_pool(name="vps", bufs=4, space="PSUM"))
        for p in range(NPAIR):
            for ch in range(4):
                sl = slice(ch * 512, (ch + 1) * 512)
                q_ps = qkps.tile([P, 512], f32, tag="q")
                k_ps = qkps.tile([P, 512], f32, tag="k")
                for cp in range(NCP):
                    nc.tensor.matmul(q_ps[:], wqk[p][:, cp, :, 0:P],
                                     xa[cp][:, :, sl], start=(cp == 0),
                                     stop=False, perf_mode=PM.DoubleRow)
                    nc.tensor.matmul(k_ps[:], wqk[p][:, cp, :, P:256],
                                     xa[cp][:, :, sl], start=(cp == 0),
                                     stop=False, perf_mode=PM.DoubleRow)
                nc.tensor.matmul(q_ps[:], ccqk[p][:, :, 0:P], xaug[:, :, sl],
                                 start=False, stop=True, perf_mode=PM.DoubleRow)
                nc.tensor.matmul(k_ps[:], ccqk[p][:, :, P:256], xaug[:, :, sl],
                                 start=False, stop=True, perf_mode=PM.DoubleRow)
                nc.vector.tensor_mul(qq[p][:, sl], q_ps[:], a1q[ch][:])
                nc.vector.tensor_copy(kk[p][:, sl], k_ps[:])
            for st in range(16):
                ssl = slice(st * P, (st + 1) * P)
                v_ps = vps.tile([P, P], f32, tag="v")
                for cp in range(NCP):
                    nc.tensor.matmul(v_ps[:], xa[cp][:, :, ssl], wv[p][:, cp],
                                     start=(cp == 0), stop=False,
                                     perf_mode=PM.DoubleRow)
                nc.tensor.matmul(v_ps[:], xaug[:, :, ssl], ccv[p][:],
                                 start=False, stop=True, perf_mode=PM.DoubleRow)
                nc.vector.tensor_scalar_mul(
                    v_aug[(p, st // 2)][:, st % 2, :, 0:64],
                    v_ps[:].rearrange("a (b c) -> a b c", b=2),
                    rvcol[:, st:st + 1])
        p2.close()
        pxin.close()

        # ===== Phase 3+4: attention (qc-major) + proj + ReduceScatter =====
        p3 = ExitStack()
        st_ps = p3.enter_context(tc.tile_pool(name="stps", bufs=2, space="PSUM"))
        att_ps = p3.enter_context(tc.tile_pool(name="attps", bufs=2, space="PSUM"))
        pj_ps = p3.enter_context(tc.tile_pool(name="pjps", bufs=2, space="PSUM"))
        sasb_pool = p3.enter_context(tc.tile_pool(name="sasb", bufs=4))
        end_pool = p3.enter_context(tc.tile_pool(name="endp", bufs=4))
        rec_pool = p3.enter_context(tc.tile_pool(name="recp", bufs=2))
        bcr_pool = p3.enter_context(tc.tile_pool(name="bcrp", bufs=2))

        for qc in QC_ORDER:
            qsl = slice(qc * 512, (qc + 1) * 512)
            n_sp = 2 * (qc + 1)
            for p in range(NPAIR):
                for hh in range(2):
                    hsl = slice(hh * 64, (hh + 1) * 64)
                    att = att_ps.tile([65, 512], f32, tag="att")
                    pend = []  # (spi, et) with exp issued, PV pending

                    def flush_pv(upto):
                        while len(pend) > upto:
                            spi_, et_ = pend.pop(0)
                            nc.tensor.matmul(
                                att[:], v_aug[(p, spi_)][:, :, hh, :], et_[:],
                                start=(spi_ == 0), stop=(spi_ == n_sp - 1),
                                perf_mode=PM.DoubleRow)

                    for spi in range(n_sp):
                        diag = (spi >= 2 * qc)
                        if diag:
                            et = e_diag[(p, hh, spi - 2 * qc)]
                        else:
                            et = end_pool.tile([P, 2, 512], f8, tag="e")
                        stp = st_ps.tile([P, 2, 512], f32, tag="st")
                        for j in range(2):
                            si = 2 * spi + j
                            ssl = slice(si * P, (si + 1) * P)
                            off = si - 4 * qc
                            nc.tensor.matmul(stp[:, j, :], kk[p][hsl, ssl],
                                             qq[p][hsl, qsl],
                                             start=True, stop=not diag)
                            if diag:
                                nc.tensor.matmul(
                                    stp[:, j, off * P:(off + 1) * P],
                                    id20[:], tril[:], start=False, stop=True,
                                    skip_group_check=True)
                        if diag:
                            for j in range(2):
                                si = 2 * spi + j
                                off = si - 4 * qc
                                nc.scalar.activation(
                                    et[0:P, j, off * P:512],
                                    stp[:, j, off * P:512], AF.Exp,
                                    bias=ebias[:], scale=rvcol[:, si:si + 1])
                        else:
                            nc.scalar.activation(
                                et[0:P, :, :], stp[:], AF.Exp, bias=ebias[:],
                                scale=rvcol[:, 2 * spi:2 * spi + 1])
                        pend.append((spi, et))
                        flush_pv(1)
                    flush_pv(0)
                    rec = rec_pool.tile([1, 512], f32, tag="rec")
                    nc.vector.reciprocal(rec[:], att[64:65, :])
                    bcr = bcr_pool.tile([64, 512], f32, tag="bcr")
                    nc.gpsimd.partition_broadcast(bcr[:], rec[:])
                    nc.vector.tensor_mul(attT[p // 2][hsl, p % 2, qsl],
                                         att[0:64, :], bcr[:])
            # proj for this quarter (both wproj split terms)
            r, fold = qc % 2, qc // 2
            for co in range(NCT):
                pp_ps = pj_ps.tile([P, 512], f32, tag="pp")
                for pp in range(2):
                    for s in range(2):
                        nc.tensor.matmul(pp_ps[:],
                                         wpj[s * 2 + pp][:, :, co * P:(co + 1) * P],
                                         attT[pp][:, :, qsl],
                                         start=(pp == 0 and s == 0),
                                         stop=(pp == 1 and s == 1),
                                         perf_mode=PM.DoubleRow)
                sasb = sasb_pool.tile([P, 512], bf16, tag="sasb")
                nc.vector.tensor_copy(sasb[:], pp_ps[:])
                nc.sync.dma_start(sa_bounce[r][fold, co], sasb[:])
            if fold == 1:
                if with_collective:
                    nc.gpsimd.collective_compute(
                        "ReduceScatter", OP.add, replica_groups=GROUPS,
                        ins=[sa_bounce[r].opt()], outs=[sa_local[r].opt()])
                else:
                    nc.sync.dma_start(sa_local[r][:], sa_bounce[r][0])
        p3.close()
        pattn.close()

        # =========== Phase 5: x2 build (Pool) + LN2 stats ===========
        px2 = ExitStack()
        x2sq_pool = px2.enter_context(tc.tile_pool(name="x2sq", bufs=1))
        x2sq = [x2sq_pool.tile([P, 2, TL], f8, name=f"x2sq{cp}", tag=f"x2sq{cp}")
                for cp in range(NCP)]
        xres_pool = px2.enter_context(tc.tile_pool(name="xres", bufs=4))

        for r in range(2):
            lsl = slice(r * 512, (r + 1) * 512)
            for co in range(NCT):
                cp, j = co // 2, co % 2
                sal = xres_pool.tile([P, 512], bf16, tag="sal")
                nc.sync.dma_start(sal[:], sa_local[r][co])
                xre = xres_pool.tile([P, 512], f32, tag="xre")
                nc.sync.dma_start(xre[:], d_xres[co][:, lsl])
                nc.gpsimd.scalar_tensor_tensor(x2sb[co][:, lsl], sal[:],
                                               1.0 / 32, xre[:],
                                               OP.mult, OP.add)
                nc.gpsimd.tensor_copy(x2a[cp][:, j, lsl], x2sb[co][:, lsl])
                nc.gpsimd.scalar_tensor_tensor(x2b[cp][:, j, lsl],
                                               x2sb[co][:, lsl], 0.0,
                                               x2a[cp][:, j, lsl],
                                               OP.add, OP.subtract)
                nc.gpsimd.tensor_mul(x2sq[cp][:, j, lsl], x2sb[co][:, lsl],
                                     x2sb[co][:, lsl])

        p5 = ExitStack()
        st_ps2 = p5.enter_context(tc.tile_pool(name="stps2", bufs=2, space="PSUM"))
        row2_pool = p5.enter_context(tc.tile_pool(name="row2", bufs=4))
        for h in range(2):
            lsl = slice(h * 512, (h + 1) * 512)
            sx = st_ps2.tile([1, 512], f32, tag="sx2")
            sq = st_ps2.tile([1, 512], f32, tag="sq2")
            for cp in range(NCP):
                nc.tensor.matmul(sx[:], ones8[:], x2a[cp][:, :, lsl],
                                 start=(cp == 0), stop=False,
                                 perf_mode=PM.DoubleRow)
                nc.tensor.matmul(sq[:], ones8[:], x2sq[cp][:, :, lsl],
                                 start=(cp == 0), stop=(cp == NCP - 1),
                                 perf_mode=PM.DoubleRow)
            for cp in range(NCP):
                nc.tensor.matmul(sx[:], ones8[:], x2b[cp][:, :, lsl],
                                 start=False, stop=(cp == NCP - 1),
                                 perf_mode=PM.DoubleRow)
            nc.scalar.activation(xaug2[0:1, 0, lsl], sx[:], AF.Copy,
                                 scale=-1.0 / C)
            mu = row2_pool.tile([1, 512], f32, tag="r")
            nc.scalar.activation(mu[:], sx[:], AF.Copy, scale=1.0 / C)
            ex2 = row2_pool.tile([1, 512], f32, tag="r")
            nc.scalar.activation(ex2[:], sq[:], AF.Copy, scale=1.0 / C)
            var = row2_pool.tile([1, 512], f32, tag="r")
            nc.vector.tensor_mul(var[:], mu[:], mu[:])
            nc.vector.scalar_tensor_tensor(var[:], ex2[:], EPS, var[:],
                                           OP.add, OP.subtract)
            std = row2_pool.tile([1, 512], f32, tag="r")
            nc.scalar.activation(std[:], var[:], AF.Sqrt)
            nc.vector.tensor_copy(xaug2[0:1, 1, lsl], std[:])
            rs2 = row2_pool.tile([1, 512], f32, tag="r")
            nc.vector.reciprocal(rs2[:], std[:])
            rs2s = row2_pool.tile([1, 512], f32, tag="r")
            nc.scalar.activation(rs2s[:], rs2[:], AF.Copy, scale=1.0 / 1024)
            nc.gpsimd.partition_broadcast(a2[h][:], rs2s[:])
        p5.close()
        px2.close()

        # =========== Phase 6: FFN ===========
        p6 = ExitStack()
        w1_pool = p6.enter_context(tc.tile_pool(name="w1", bufs=2))
        cc1_pool = p6.enter_context(tc.tile_pool(name="cc1", bufs=2))
        relu_pool = p6.enter_context(tc.tile_pool(name="relu", bufs=1))
        ffn_ps = p6.enter_context(tc.tile_pool(name="ffnps", bufs=4, space="PSUM"))
        relu = [relu_pool.tile([P, 2, TL], f8, name=f"rl{fq}", tag=f"rl{fq}")
                for fq in range(NFQ)]

        for fog in range(8):
            gsl = slice(fog * 512, (fog + 1) * 512)
            w1t = []
            for sp_ in range(2):
                for cp in range(NCP):
                    w = w1_pool.tile([P, 2, 512], f8, tag=f"w1_{sp_}{cp}")
                    nc.sync.dma_start(w[:], d_w1[sp_, cp][:, :, gsl])
                    w1t.append(w)
            cc1 = cc1_pool.tile([1, 2, 512], f8, tag="cc1")
            nc.sync.dma_start(cc1[:], d_cc1[:, gsl].unsqueeze(0))
            for fol in range(4):
                fo = fog * 4 + fol
                fsl = slice(fol * P, (fol + 1) * P)
                fq, fj = fo // 2, fo % 2
                for h in range(2):
                    lsl = slice(h * 512, (h + 1) * 512)
                    fp = ffn_ps.tile([P, 512], f32, tag="fp")
                    for cp in range(NCP):
                        nc.tensor.matmul(fp[:], w1t[cp][:, :, fsl],
                                         x2a[cp][:, :, lsl], start=(cp == 0),
                                         stop=False, perf_mode=PM.DoubleRow)
                    for cp in range(NCP):
                        nc.tensor.matmul(fp[:], w1t[NCP + cp][:, :, fsl],
                                         x2a[cp][:, :, lsl], start=False,
                                         stop=False, perf_mode=PM.DoubleRow)
                    for cp in range(NCP):
                        nc.tensor.matmul(fp[:], w1t[cp][:, :, fsl],
                                         x2b[cp][:, :, lsl], start=False,
                                         stop=False, perf_mode=PM.DoubleRow)
                    nc.tensor.matmul(fp[:], cc1[:, :, fsl], xaug2[:, :, lsl],
                                     start=False, stop=True,
                                     perf_mode=PM.DoubleRow)
                    if fo % 2 == 0:
                        nc.scalar.activation(relu[fq][:, fj, lsl], fp[:], AF.Relu)
                    else:
                        nc.vector.tensor_scalar_max(relu[fq][:, fj, lsl], fp[:],
                                                    0.0)

        w2_pool = p6.enter_context(tc.tile_pool(name="w2", bufs=4))
        out_pool = p6.enter_context(tc.tile_pool(name="outsb", bufs=4))
        b2_pool = p6.enter_context(tc.tile_pool(name="b2p", bufs=1))
        b2col = [b2_pool.tile([P, 1], f32, name=f"b2c{co}", tag=f"b2c{co}")
                 for co in range(NCT)]
        for co in range(NCT):
            nc.sync.dma_start(b2col[co][:], d_b2[co].unsqueeze(1))
        for co in range(NCT):
            w2a = w2_pool.tile([P, NFQ, 2, P], f8, tag="w2t")
            nc.sync.dma_start(w2a[:], d_w2[0, co])
            w2b = w2_pool.tile([P, NFQ, 2, P], f8, tag="w2t")
            nc.sync.dma_start(w2b[:], d_w2[1, co])
            for h in range(2):
                lsl = slice(h * 512, (h + 1) * 512)
                fp = ffn_ps.tile([P, 512], f32, tag="fp2")
                for fq in range(NFQ):
                    nc.tensor.matmul(fp[:], w2a[:, fq], relu[fq][:, :, lsl],
                                     start=(fq == 0), stop=False,
                                     perf_mode=PM.DoubleRow)
                for fq in range(NFQ):
                    nc.tensor.matmul(fp[:], w2b[:, fq], relu[fq][:, :, lsl],
                                     start=False, stop=(fq == NFQ - 1),
                                     perf_mode=PM.DoubleRow)
                tmp = out_pool.tile([P, 512], f32, tag="tmp")
                nc.vector.tensor_mul(tmp[:], fp[:], a2[h][:])
                osb = out_pool.tile([P, 512], f32, tag="osb")
                nc.vector.scalar_tensor_tensor(osb[:], tmp[:], b2col[co][:],
                                               x2sb[co][:, lsl], OP.add, OP.add)
                nc.sync.dma_start(d_out[co][:, lsl], osb[:])
        p6.close()

    nc.compile()
    return nc


def _q8(v):
    return np.asarray(v, np.float32).astype(E4)


def _prep_inputs(x, Wq, Wk, Wv, Wproj, bproj, W1, b1, W2, b2, g1, beta1, g2,
                 beta2):
    f32 = np.float32
    scale = HS ** -0.5
    x = np.asarray(x, f32)
    Wq = np.asarray(Wq, f32); Wk = np.asarray(Wk, f32); Wv = np.asarray(Wv, f32)
    Wproj = np.asarray(Wproj, f32); bproj = np.asarray(bproj, f32)
    W1 = np.asarray(W1, f32); b1 = np.asarray(b1, f32)
    W2 = np.asarray(W2, f32); b2 = np.asarray(b2, f32)
    g1 = np.asarray(g1, f32); beta1 = np.asarray(beta1, f32)
    g2 = np.asarray(g2, f32); beta2 = np.asarray(beta2, f32)

    # ---- shared (g-independent) weights ----
    w1s = (g2[:, None] * W1) * 32.0                       # [C, F]
    w1a = _q8(w1s)
    w1b = _q8(w1s - w1a.astype(f32))
    w1_pack = np.stack([w1a.reshape(NCP, 2, P, F).transpose(0, 2, 1, 3),
                        w1b.reshape(NCP, 2, P, F).transpose(0, 2, 1, 3)])
    cc1 = np.empty((2, F), f32)
    cc1[0] = (w1a.astype(f32) + w1b.astype(f32)).sum(0)
    cc1[1] = (b1 + beta2 @ W1) * 32.0
    cc1 = _q8(cc1)

    w2s = W2 * 32.0                                        # [F, C]
    w2a = _q8(w2s)
    w2b = _q8(w2s - w2a.astype(f32))
    # [2, NCT, P, NFQ, 2, P]: w2[s, co, p, fq, j, cc] = w2s[(2fq+j)*128+p, co*128+cc]
    def pack_w2(w):
        return np.ascontiguousarray(
            w.reshape(NFQ, 2, P, NCT, P).transpose(3, 2, 0, 1, 4))
    w2_pack = np.stack([pack_w2(w2a), pack_w2(w2b)])

    b2r = b2.reshape(NCT, P)
    kp = np.arange(P)[:, None]
    lq = np.arange(P)[None, :]
    trilneg = (-240.0 * (lq < kp)).astype(E4)
    ident20 = (20.0 * np.eye(P)).astype(E4)

    # ---- per-batch x ----
    xa_b, xsq_b, xresT_b = [], [], []
    for b in range(B):
        xT = np.ascontiguousarray(x[b].T)                  # [C, T]
        xq = _q8(xT)
        xa_b.append(np.ascontiguousarray(
            xq.reshape(NCP, 2, P, T).transpose(0, 2, 1, 3)))
        xsq_b.append(np.ascontiguousarray(
            _q8(xq.astype(f32) ** 2).reshape(NCP, 2, P, T).transpose(0, 2, 1, 3)))
        xresT_b.append(xT + bproj[:, None])                # bproj folded in

    # ---- per-group attention weights ----
    per_g = {}
    for g in range(2):
        wqk = np.empty((NPAIR, P, NCP, 2, 256), E4)
        ccqk = np.empty((NPAIR, 2, 256), f32)
        wv_ = np.empty((NPAIR, P, NCP, 2, P), E4)
        ccv = np.empty((NPAIR, 2, P), f32)
        for p in range(NPAIR):
            hA, hB = g * 8 + 2 * p, g * 8 + 2 * p + 1
            for (Wfull, scl, col) in ((Wq, scale * 16.0, slice(0, P)),
                                      (Wk, 32.0, slice(P, 256))):
                wt = np.concatenate([Wfull[hA], Wfull[hB]], axis=1) * scl  # [C,128]
                wq8 = _q8(g1[:, None] * wt)
                wqk[p, :, :, :, col] = wq8.reshape(NCP, 2, P, P).transpose(
                    2, 0, 1, 3)
                ccqk[p, 0, col] = wq8.astype(f32).sum(0)
                ccqk[p, 1, col] = beta1 @ wt
            wt = np.concatenate([Wv[hA], Wv[hB]], axis=1) * 32.0
            wq8 = _q8(g1[:, None] * wt)
            wv_[p] = wq8.reshape(NCP, 2, P, P).transpose(2, 0, 1, 3)
            ccv[p, 0] = wq8.astype(f32).sum(0)
            ccv[p, 1] = beta1 @ wt
        # wpj [2(split), 2(pp), P, 2(j), C]: wps[pp, j, f, c] -> [pp, f, j, c]
        wps = np.ascontiguousarray(
            Wproj[g * 512:(g + 1) * 512] * 32.0).reshape(2, 2, P, C)
        wpa = _q8(wps)
        wpb = _q8(wps - wpa.astype(f32))
        wpj = np.stack([wpa.transpose(0, 2, 1, 3), wpb.transpose(0, 2, 1, 3)])
        per_g[g] = dict(wqk=wqk, ccqk=_q8(ccqk), wv=wv_, ccv=_q8(ccv),
                        wpj=np.ascontiguousarray(wpj))

    in_maps = []
    for c in range(8):
        b, g = c // 2, c % 2
        m = dict(
            xa=xa_b[b], xsq=xsq_b[b],
            xres=np.ascontiguousarray(
                xresT_b[b][:, g * TL:(g + 1) * TL].reshape(NCT, P, TL)),
            w1=w1_pack, cc1=cc1, w2=w2_pack, b2=b2r,
            trilneg=trilneg, ident20=ident20,
        )
        m.update(per_g[g])
        in_maps.append(m)
    return in_maps


def kernel(**inputs):
    from concourse.bass_utils import run_bass_kernel_spmd

    if "nc" not in _CACHE:
        _CACHE["nc"] = _build(with_collective=True)
    nc = _CACHE["nc"]
    in_maps = _prep_inputs(**inputs)
    res = None
    last_err = None
    for _attempt in range(3):
        try:
            res = run_bass_kernel_spmd(nc, in_maps, list(range(8)))
            break
        except Exception as e:  # transient runtime/tunnel hiccups
            last_err = e
            import time
            time.sleep(10)
    if res is None:
        raise last_err
    out = np.empty((B, T, C), np.float32)
    for c in range(8):
        b, g = c // 2, c % 2
        outT = res.results[c]["outT"].reshape(C, TL)
        out[b, g * TL:(g + 1) * TL, :] = outT.T
    return out


# revision 10
# speedup vs baseline: 1.5186x; 1.0863x over previous
"""Trainium2 Bass kernel for a dense transformer block (pre-LN, causal MHA + FFN).

Sharding (8 NeuronCores): core c = 2*b + g handles sequence b (of B=4) and
half g (of 2): tensor-parallel attention over 8 of 16 heads (partial proj,
pairwise ReduceScatter over {2b, 2b+1}), then token-parallel LN2+FFN over
its 1024 of 2048 tokens.

Matmul strategy: fp8-e4m3 DoubleRow matmuls (0.5 cyc/row) everywhere except
the attention-score matmul (bf16). Accuracy-critical operands use 2-term fp8
splits (value + fp8 residual): Wproj, W1, W2 (host-side) and x2 (device-side,
Pool engine). LayerNorms are folded into the matmuls via augmented [1,2,*]
DoubleRow correction matmuls (colsum x (-mu), beta-dot x std) with per-column
rstd applied at eviction (Q, V) or inside exp (K, via activation scale).
Causal masking is done by accumulating ident20^T @ trilneg = -4800 onto the
diagonal stair blocks of S before exp (survives the ~1/32 exp scale).
"""
import numpy as np
import ml_dtypes
from contextlib import ExitStack

B, T, C = 4, 2048, 1024
H, HS = 16, 64
F = 4 * C
P = 128
EPS = 1e-5
NCT = C // P        # 8 c-tiles
NCP = NCT // 2      # 4 c-tile pairs
NPAIR = 4           # head-pairs per core
TL = T // 2         # 1024 local tokens
NFQ = 16            # f-tile pairs (FFN hidden 4096 = 32 tiles = 16 pairs)
GROUPS = [[0, 1], [2, 3], [4, 5], [6, 7]]
QC_ORDER = [0, 2, 1, 3]   # quarters 0,2 feed RS0; 1,3 feed RS1

E4 = ml_dtypes.float8_e4m3
BF = ml_dtypes.bfloat16

_CACHE = {}


def _build(with_collective=True):
    import concourse.tile as tile
    from concourse import bacc, mybir

    f32 = mybir.dt.float32
    f8 = mybir.dt.float8e4
    bf16 = mybir.dt.bfloat16
    AF = mybir.ActivationFunctionType
    OP = mybir.AluOpType
    PM = mybir.MatmulPerfMode

    nc = bacc.Bacc("TRN2", target_bir_lowering=False, debug=False, num_devices=8)

    # ---- DRAM I/O ----
    d_xa = nc.dram_tensor("xa", [P, NCP, 2, T], f8, kind="ExternalInput").ap()
    d_xsq = nc.dram_tensor("xsq", [P, NCP, 2, T], f8, kind="ExternalInput").ap()
    d_xres = nc.dram_tensor("xres", [NCT, P, TL], f32, kind="ExternalInput").ap()
    d_wqk = nc.dram_tensor("wqk", [P, NPAIR, NCP, 2, 256], f8,
                           kind="ExternalInput").ap()
    d_ccqk = nc.dram_tensor("ccqk", [NPAIR, 2, 256], f8, kind="ExternalInput").ap()
    d_wv = nc.dram_tensor("wv", [P, NPAIR, NCP, 2, P], f8,
                          kind="ExternalInput").ap()
    d_ccv = nc.dram_tensor("ccv", [NPAIR, 2, P], f8, kind="ExternalInput").ap()
    d_wpj = nc.dram_tensor("wpj", [P, 2, 2, 2, C], f8, kind="ExternalInput").ap()
    d_w1 = nc.dram_tensor("w1", [2, NCP, P, 2, F], f8, kind="ExternalInput").ap()
    d_cc1 = nc.dram_tensor("cc1", [2, F], f8, kind="ExternalInput").ap()
    d_w2 = nc.dram_tensor("w2", [2, NCT, P, NFQ, 2, P], f8,
                          kind="ExternalInput").ap()
    d_b2 = nc.dram_tensor("b2", [NCT, P], f32, kind="ExternalInput").ap()
    d_tril = nc.dram_tensor("trilneg", [P, P], f8, kind="ExternalInput").ap()
    d_id20 = nc.dram_tensor("ident20", [P, P], f8, kind="ExternalInput").ap()
    d_out = nc.dram_tensor("outT", [NCT, P, TL], f32, kind="ExternalOutput").ap()

    with tile.TileContext(nc) as tc, ExitStack() as ctx:
        dram = ctx.enter_context(tc.tile_pool(name="dram", bufs=1, space="DRAM"))
        sa_bounce = [dram.tile([2, NCT, P, 512], bf16, name=f"sab{r}")
                     for r in range(2)]
        sa_local = [dram.tile([NCT, P, 512], bf16, name=f"sal{r}")
                    for r in range(2)]

        # ---- persistent constants / inputs ----
        const = ctx.enter_context(tc.tile_pool(name="const", bufs=1))
        tril = const.tile([P, P], f8)
        nc.sync.dma_start(tril[:], d_tril[:])
        id20 = const.tile([P, P], f8)
        nc.sync.dma_start(id20[:], d_id20[:])
        ones8 = const.tile([P, 2, 1], f8)
        nc.vector.memset(ones8[:], 1.0)
        ones11 = const.tile([1, 1], f32)
        nc.vector.memset(ones11[:], 1.0)
        ebias = const.tile([P, 1], f32)
        nc.vector.memset(ebias[:], -2.0)

        wat_pool = ctx.enter_context(tc.tile_pool(name="watt", bufs=1))
        wqk_t = wat_pool.tile([P, NPAIR, NCP, 2, 256], f8, tag="wqk")
        nc.sync.dma_start(wqk_t[:], d_wqk[:])
        wqk = [wqk_t[:, p] for p in range(NPAIR)]
        ccqk_t = wat_pool.tile([1, NPAIR, 2, 256], f8, tag="ccqk")
        nc.sync.dma_start(ccqk_t[:], d_ccqk[:].unsqueeze(0))
        ccqk = [ccqk_t[:, p] for p in range(NPAIR)]
        wv_t = wat_pool.tile([P, NPAIR, NCP, 2, P], f8, tag="wv")
        nc.sync.dma_start(wv_t[:], d_wv[:])
        wv = [wv_t[:, p] for p in range(NPAIR)]
        ccv_t = wat_pool.tile([1, NPAIR, 2, P], f8, tag="ccv")
        nc.sync.dma_start(ccv_t[:], d_ccv[:].unsqueeze(0))
        ccv = [ccv_t[:, p] for p in range(NPAIR)]
        wpj_t = wat_pool.tile([P, 2, 2, 2, C], f8, tag="wpj")
        nc.sync.dma_start(wpj_t[:], d_wpj[:])
        wpj = [wpj_t[:, s, pp] for s in range(2) for pp in range(2)]

        # persistent row tiles (partition 0)
        rows_pool = ctx.enter_context(tc.tile_pool(name="rows", bufs=1))
        xaug = rows_pool.tile([1, 2, T], f8, tag="xaug")       # (-mu, std)
        xaug2 = rows_pool.tile([1, 2, TL], f8, tag="xaug2")

        bc_pool = ctx.enter_context(tc.tile_pool(name="bc", bufs=1))
        a1q = [bc_pool.tile([P, 512], f32, name=f"a1q{ch}", tag=f"a1q{ch}")
               for ch in range(4)]
        a2 = [bc_pool.tile([P, 512], f32, name=f"a2_{h}", tag=f"a2_{h}")
              for h in range(2)]
        rcol_pool = ctx.enter_context(tc.tile_pool(name="rcol", bufs=1))
        rvcol = rcol_pool.tile([P, 16], f32, tag="rvcol")      # rstd/32 by stripe
        # pin allocation order (pool frees must be LIFO vs first-use order)
        for t_ in (xaug, xaug2):
            nc.gpsimd.memset(t_[0:1, 0:1], 0.0)
        for t_ in a1q + a2 + [rvcol]:
            nc.gpsimd.memset(t_[:, 0:1], 0.0)

        x2_pool = ctx.enter_context(tc.tile_pool(name="x2f", bufs=1))
        x2sb = [x2_pool.tile([P, TL], f32, name=f"x2sb{co}", tag=f"x2sb{co}")
                for co in range(NCT)]
        x2q_pool = ctx.enter_context(tc.tile_pool(name="x2q", bufs=1))
        x2a = [x2q_pool.tile([P, 2, TL], f8, name=f"x2a{cp}", tag=f"x2a{cp}")
               for cp in range(NCP)]
        x2b = [x2q_pool.tile([P, 2, TL], f8, name=f"x2b{cp}", tag=f"x2b{cp}")
               for cp in range(NCP)]
        for t_ in x2sb + x2a + x2b:
            nc.gpsimd.memset(t_[:, 0:1], 0.0)

        # attention working tiles (freed after proj)
        pattn = ExitStack()
        qk_pool = pattn.enter_context(tc.tile_pool(name="qk", bufs=1))
        qq = [qk_pool.tile([P, T], bf16, name=f"qq{p}", tag=f"qq{p}")
              for p in range(NPAIR)]
        kk = [qk_pool.tile([P, T], bf16, name=f"kk{p}", tag=f"kk{p}")
              for p in range(NPAIR)]
        for t_ in qq + kk:
            nc.gpsimd.memset(t_[:, 0:1], 0.0)
        va_pool = pattn.enter_context(tc.tile_pool(name="va", bufs=1))
        v_aug = {}
        for p in range(NPAIR):
            for sp in range(8):
                va = va_pool.tile([P, 2, 2, 65], f8, name=f"va{p}_{sp}",
                                  tag=f"va{p}_{sp}")
                nc.gpsimd.memset(va[:, :, :, 64:65], 1.0)
                v_aug[(p, sp)] = va
        ediag_pool = pattn.enter_context(tc.tile_pool(name="ediag", bufs=1))
        e_diag = {}
        for p in range(NPAIR):
            for hh in range(2):
                for di in range(2):
                    et = ediag_pool.tile([P, 2, 512], f8,
                                         name=f"ed{p}_{hh}_{di}",
                                         tag=f"ed{p}_{hh}_{di}")
                    for j in range(2):
                        z = (2 * di + j) * P
                        if z:
                            nc.gpsimd.memset(et[:, j, 0:z], 0.0)
                    e_diag[(p, hh, di)] = et
        attT_pool = pattn.enter_context(tc.tile_pool(name="attT", bufs=1))
        attT = [attT_pool.tile([P, 2, T], f8, name=f"attT{pp}", tag=f"attT{pp}")
                for pp in range(2)]
        for pp in range(2):
            nc.gpsimd.memset(attT[pp][:, :, 0:1], 0.0)  # pin alloc order

        pxin = ExitStack()
        xin_pool = pxin.enter_context(tc.tile_pool(name="xin", bufs=1))
        xa_t = xin_pool.tile([P, NCP, 2, T], f8, tag="xa")
        nc.sync.dma_start(xa_t[:], d_xa[:])
        xa = [xa_t[:, cp] for cp in range(NCP)]
        pxsq = ExitStack()
        xsq_pool = pxsq.enter_context(tc.tile_pool(name="xsq", bufs=1))
        xsq_t = xsq_pool.tile([P, NCP, 2, T], f8, tag="xsq")
        nc.sync.dma_start(xsq_t[:], d_xsq[:])
        xsq = [xsq_t[:, cp] for cp in range(NCP)]

        # =========== Phase 1: LN1 stats ===========
        p1 = ExitStack()
        st_ps1 = p1.enter_context(tc.tile_pool(name="stps1", bufs=2, space="PSUM"))
        row1_pool = p1.enter_context(tc.tile_pool(name="row1", bufs=6))
        for ch in range(4):
            sl = slice(ch * 512, (ch + 1) * 512)
            sx = st_ps1.tile([1, 512], f32, tag="sx")
            sq = st_ps1.tile([1, 512], f32, tag="sq")
            for cp in range(NCP):
                nc.tensor.matmul(sx[:], ones8[:], xa[cp][:, :, sl],
                                 start=(cp == 0), stop=(cp == NCP - 1),
                                 perf_mode=PM.DoubleRow)
                nc.tensor.matmul(sq[:], ones8[:], xsq[cp][:, :, sl],
                                 start=(cp == 0), stop=(cp == NCP - 1),
                                 perf_mode=PM.DoubleRow)
            # -mu (fp8 aug row) and f32 rows
            nc.scalar.activation(xaug[0:1, 0, sl], sx[:], AF.Copy, scale=-1.0 / C)
            mu = row1_pool.tile([1, 512], f32, tag="r")
            nc.scalar.activation(mu[:], sx[:], AF.Copy, scale=1.0 / C)
            ex2 = row1_pool.tile([1, 512], f32, tag="r")
            nc.scalar.activation(ex2[:], sq[:], AF.Copy, scale=1.0 / C)
            var = row1_pool.tile([1, 512], f32, tag="r")
            nc.vector.tensor_mul(var[:], mu[:], mu[:])
            nc.vector.scalar_tensor_tensor(var[:], ex2[:], EPS, var[:],
                                           OP.add, OP.subtract)
            std = row1_pool.tile([1, 512], f32, tag="r")
            nc.scalar.activation(std[:], var[:], AF.Sqrt)
            nc.vector.tensor_copy(xaug[0:1, 1, sl], std[:])
            rstd = row1_pool.tile([1, 512], f32, tag="r")
            nc.vector.reciprocal(rstd[:], std[:])
            r16 = row1_pool.tile([1, 512], f32, tag="r")
            nc.scalar.activation(r16[:], rstd[:], AF.Copy, scale=1.0 / 16)
            r32 = row1_pool.tile([1, 512], f32, tag="r")
            nc.scalar.activation(r32[:], rstd[:], AF.Copy, scale=1.0 / 32)
            nc.gpsimd.partition_broadcast(a1q[ch][:], r16[:])
            # rstd/32 per-stripe columns via mini PE transposes
            rc_ps = st_ps1.tile([P, 4], f32, tag="rc")
            for si in range(4):
                nc.tensor.transpose(rc_ps[:, si:si + 1],
                                    r32[:, si * P:(si + 1) * P], ones11[:])
            nc.vector.tensor_copy(rvcol[:, ch * 4:(ch + 1) * 4], rc_ps[:])
        p1.close()
        pxsq.close()

        # =========== Phase 2: QKV (all pairs) ===========
        p2 = ExitStack()
        qkps = p2.enter_context(tc.tile_pool(name="qkps", bufs=2, space="PSUM"))
        vps = p2.enter_context(tc.tile_pool(name="vps", bufs=4, space="PSUM"))
        for p in range(NPAIR):
            for ch in range(4):
                sl = slice(ch * 512, (ch + 1) * 512)
                q_ps = qkps.tile([P, 512], f32, tag="q")
                k_ps = qkps.tile([P, 512], f32, tag="k")
                for cp in range(NCP):
                    nc.tensor.matmul(q_ps[:], wqk[p][:, cp, :, 0:P],
                                     xa[cp][:, :, sl], start=(cp == 0),
                                     stop=False, perf_mode=PM.DoubleRow)
                    nc.tensor.matmul(k_ps[:], wqk[p][:, cp, :, P:256],
                                     xa[cp][:, :, sl], start=(cp == 0),
                                     stop=False, perf_mode=PM.DoubleRow)
                nc.tensor.matmul(q_ps[:], ccqk[p][:, :, 0:P], xaug[:, :, sl],
                                 start=False, stop=True, perf_mode=PM.DoubleRow)
                nc.tensor.matmul(k_ps[:], ccqk[p][:, :, P:256], xaug[:, :, sl],
                                 start=False, stop=True, perf_mode=PM.DoubleRow)
                nc.vector.tensor_mul(qq[p][:, sl], q_ps[:], a1q[ch][:])
                nc.scalar.copy(kk[p][:, sl], k_ps[:])
            for st in range(16):
                ssl = slice(st * P, (st + 1) * P)
                v_ps = vps.tile([P, P], f32, tag="v")
                for cp in range(NCP):
                    nc.tensor.matmul(v_ps[:], xa[cp][:, :, ssl], wv[p][:, cp],
                                     start=(cp == 0), stop=False,
                                     perf_mode=PM.DoubleRow)
                nc.tensor.matmul(v_ps[:], xaug[:, :, ssl], ccv[p][:],
                                 start=False, stop=True, perf_mode=PM.DoubleRow)
                if st % 2 == 0:
                    nc.vector.tensor_scalar_mul(
                        v_aug[(p, st // 2)][:, st % 2, :, 0:64],
                        v_ps[:].rearrange("a (b c) -> a b c", b=2),
                        rvcol[:, st:st + 1])
                else:
                    nc.scalar.activation(
                        v_aug[(p, st // 2)][:, st % 2, :, 0:64],
                        v_ps[:].rearrange("a (b c) -> a b c", b=2),
                        AF.Copy, scale=rvcol[:, st:st + 1])
        p2.close()
        pxin.close()

        # ===== Phase 3+4: attention (qc-major) + proj + ReduceScatter =====
        p3 = ExitStack()
        st_ps = p3.enter_context(tc.tile_pool(name="stps", bufs=2, space="PSUM"))
        att_ps = p3.enter_context(tc.tile_pool(name="attps", bufs=2, space="PSUM"))
        pj_ps = p3.enter_context(tc.tile_pool(name="pjps", bufs=2, space="PSUM"))
        sasb_pool = p3.enter_context(tc.tile_pool(name="sasb", bufs=4))
        end_pool = p3.enter_context(tc.tile_pool(name="endp", bufs=4))
        rec_pool = p3.enter_context(tc.tile_pool(name="recp", bufs=2))
        bcr_pool = p3.enter_context(tc.tile_pool(name="bcrp", bufs=2))

        for qc in QC_ORDER:
            qsl = slice(qc * 512, (qc + 1) * 512)
            n_sp = 2 * (qc + 1)
            for p in range(NPAIR):
                for hh in range(2):
                    hsl = slice(hh * 64, (hh + 1) * 64)
                    att = att_ps.tile([65, 512], f32, tag="att")
                    pend = []  # (spi, et) with exp issued, PV pending

                    def flush_pv(upto):
                        while len(pend) > upto:
                            spi_, et_ = pend.pop(0)
                            nc.tensor.matmul(
                                att[:], v_aug[(p, spi_)][:, :, hh, :], et_[:],
                                start=(spi_ == 0), stop=(spi_ == n_sp - 1),
                                perf_mode=PM.DoubleRow)

                    for spi in range(n_sp):
                        diag = (spi >= 2 * qc)
                        if diag:
                            et = e_diag[(p, hh, spi - 2 * qc)]
                        else:
                            et = end_pool.tile([P, 2, 512], f8, tag="e")
                        stp = st_ps.tile([P, 2, 512], f32, tag="st")
                        for j in range(2):
                            si = 2 * spi + j
                            ssl = slice(si * P, (si + 1) * P)
                            off = si - 4 * qc
                            nc.tensor.matmul(stp[:, j, :], kk[p][hsl, ssl],
                                             qq[p][hsl, qsl],
                                             start=True, stop=not diag)
                            if diag:
                                nc.tensor.matmul(
                                    stp[:, j, off * P:(off + 1) * P],
                                    id20[:], tril[:], start=False, stop=True,
                                    skip_group_check=True)
                        if diag:
                            for j in range(2):
                                si = 2 * spi + j
                                off = si - 4 * qc
                                nc.scalar.activation(
                                    et[0:P, j, off * P:512],
                                    stp[:, j, off * P:512], AF.Exp,
                                    bias=ebias[:], scale=rvcol[:, si:si + 1])
                        else:
                            nc.scalar.activation(
                                et[0:P, :, :], stp[:], AF.Exp, bias=ebias[:],
                                scale=rvcol[:, 2 * spi:2 * spi + 1])
                        pend.append((spi, et))
                        flush_pv(1)
                    flush_pv(0)
                    rec = rec_pool.tile([1, 512], f32, tag="rec")
                    nc.vector.reciprocal(rec[:], att[64:65, :])
                    bcr = bcr_pool.tile([64, 512], f32, tag="bcr")
                    nc.gpsimd.partition_broadcast(bcr[:], rec[:])
                    nc.vector.tensor_mul(attT[p // 2][hsl, p % 2, qsl],
                                         att[0:64, :], bcr[:])
            # proj for this quarter (both wproj split terms)
            r, fold = qc % 2, qc // 2
            for co in range(NCT):
                pp_ps = pj_ps.tile([P, 512], f32, tag="pp")
                for pp in range(2):
                    for s in range(2):
                        nc.tensor.matmul(pp_ps[:],
                                         wpj[s * 2 + pp][:, :, co * P:(co + 1) * P],
                                         attT[pp][:, :, qsl],
                                         start=(pp == 0 and s == 0),
                                         stop=(pp == 1 and s == 1),
                                         perf_mode=PM.DoubleRow)
                sasb = sasb_pool.tile([P, 512], bf16, tag="sasb")
                nc.vector.tensor_copy(sasb[:], pp_ps[:])
                nc.sync.dma_start(sa_bounce[r][fold, co], sasb[:])
            if fold == 1:
                if with_collective:
                    nc.gpsimd.collective_compute(
                        "ReduceScatter", OP.add, replica_groups=GROUPS,
                        ins=[sa_bounce[r].opt()], outs=[sa_local[r].opt()])
                else:
                    nc.sync.dma_start(sa_local[r][:], sa_bounce[r][0])
        p3.close()
        pattn.close()

        # =========== Phase 5: x2 build (Pool) + LN2 stats ===========
        px2 = ExitStack()
        x2sq_pool = px2.enter_context(tc.tile_pool(name="x2sq", bufs=1))
        x2sq = [x2sq_pool.tile([P, 2, TL], f8, name=f"x2sq{cp}", tag=f"x2sq{cp}")
                for cp in range(NCP)]
        xres_pool = px2.enter_context(tc.tile_pool(name="xres", bufs=4))

        for r in range(2):
            lsl = slice(r * 512, (r + 1) * 512)
            for co in range(NCT):
                cp, j = co // 2, co % 2
                sal = xres_pool.tile([P, 512], bf16, tag="sal")
                nc.sync.dma_start(sal[:], sa_local[r][co])
                xre = xres_pool.tile([P, 512], f32, tag="xre")
                nc.sync.dma_start(xre[:], d_xres[co][:, lsl])
                nc.vector.scalar_tensor_tensor(x2sb[co][:, lsl], sal[:],
                                               1.0 / 32, xre[:],
                                               OP.mult, OP.add)
                nc.scalar.copy(x2a[cp][:, j, lsl], x2sb[co][:, lsl])
                nc.vector.scalar_tensor_tensor(x2b[cp][:, j, lsl],
                                               x2sb[co][:, lsl], 0.0,
                                               x2a[cp][:, j, lsl],
                                               OP.add, OP.subtract)
                nc.scalar.square(x2sq[cp][:, j, lsl], x2sb[co][:, lsl])

        p5 = ExitStack()
        st_ps2 = p5.enter_context(tc.tile_pool(name="stps2", bufs=2, space="PSUM"))
        row2_pool = p5.enter_context(tc.tile_pool(name="row2", bufs=4))
        for h in range(2):
            lsl = slice(h * 512, (h + 1) * 512)
            sx = st_ps2.tile([1, 512], f32, tag="sx2")
            sq = st_ps2.tile([1, 512], f32, tag="sq2")
            for cp in range(NCP):
                nc.tensor.matmul(sx[:], ones8[:], x2a[cp][:, :, lsl],
                                 start=(cp == 0), stop=False,
                                 perf_mode=PM.DoubleRow)
                nc.tensor.matmul(sq[:], ones8[:], x2sq[cp][:, :, lsl],
                                 start=(cp == 0), stop=(cp == NCP - 1),
                                 perf_mode=PM.DoubleRow)
            for cp in range(NCP):
                nc.tensor.matmul(sx[:], ones8[:], x2b[cp][:, :, lsl],
                                 start=False, stop=(cp == NCP - 1),
                                 perf_mode=PM.DoubleRow)
            nc.scalar.activation(xaug2[0:1, 0, lsl], sx[:], AF.Copy,
                                 scale=-1.0 / C)
            mu = row2_pool.tile([1, 512], f32, tag="r")
            nc.scalar.activation(mu[:], sx[:], AF.Copy, scale=1.0 / C)
            ex2 = row2_pool.tile([1, 512], f32, tag="r")
            nc.scalar.activation(ex2[:], sq[:], AF.Copy, scale=1.0 / C)
            var = row2_pool.tile([1, 512], f32, tag="r")
            nc.vector.tensor_mul(var[:], mu[:], mu[:])
            nc.vector.scalar_tensor_tensor(var[:], ex2[:], EPS, var[:],
                                           OP.add, OP.subtract)
            std = row2_pool.tile([1, 512], f32, tag="r")
            nc.scalar.activation(std[:], var[:], AF.Sqrt)
            nc.vector.tensor_copy(xaug2[0:1, 1, lsl], std[:])
            rs2 = row2_pool.tile([1, 512], f32, tag="r")
            nc.vector.reciprocal(rs2[:], std[:])
            rs2s = row2_pool.tile([1, 512], f32, tag="r")
            nc.scalar.activation(rs2s[:], rs2[:], AF.Copy, scale=1.0 / 1024)
            nc.gpsimd.partition_broadcast(a2[h][:], rs2s[:])
        p5.close()
        px2.close()

        # =========== Phase 6: FFN ===========
        p6 = ExitStack()
        w1_pool = p6.enter_context(tc.tile_pool(name="w1", bufs=2))
        cc1_pool = p6.enter_context(tc.tile_pool(name="cc1", bufs=2))
        relu_pool = p6.enter_context(tc.tile_pool(name="relu", bufs=1))
        ffn_ps = p6.enter_context(tc.tile_pool(name="ffnps", bufs=4, space="PSUM"))
        relu = [relu_pool.tile([P, 2, TL], f8, name=f"rl{fq}", tag=f"rl{fq}")
                for fq in range(NFQ)]

        for fog in range(8):
            gsl = slice(fog * 512, (fog + 1) * 512)
            w1t = []
            for sp_ in range(2):
                for cp in range(NCP):
                    w = w1_pool.tile([P, 2, 512], f8, tag=f"w1_{sp_}{cp}")
                    nc.sync.dma_start(w[:], d_w1[sp_, cp][:, :, gsl])
                    w1t.append(w)
            cc1 = cc1_pool.tile([1, 2, 512], f8, tag="cc1")
            nc.sync.dma_start(cc1[:], d_cc1[:, gsl].unsqueeze(0))
            for fol in range(4):
                fo = fog * 4 + fol
                fsl = slice(fol * P, (fol + 1) * P)
                fq, fj = fo // 2, fo % 2
                for h in range(2):
                    lsl = slice(h * 512, (h + 1) * 512)
                    fp = ffn_ps.tile([P, 512], f32, tag="fp")
                    for cp in range(NCP):
                        nc.tensor.matmul(fp[:], w1t[cp][:, :, fsl],
                                         x2a[cp][:, :, lsl], start=(cp == 0),
                                         stop=False, perf_mode=PM.DoubleRow)
                    for cp in range(NCP):
                        nc.tensor.matmul(fp[:], w1t[NCP + cp][:, :, fsl],
                                         x2a[cp][:, :, lsl], start=False,
                                         stop=False, perf_mode=PM.DoubleRow)
                    for cp in range(NCP):
                        nc.tensor.matmul(fp[:], w1t[cp][:, :, fsl],
                                         x2b[cp][:, :, lsl], start=False,
                                         stop=False, perf_mode=PM.DoubleRow)
                    nc.tensor.matmul(fp[:], cc1[:, :, fsl], xaug2[:, :, lsl],
                                     start=False, stop=True,
                                     perf_mode=PM.DoubleRow)
                    if fo % 2 == 0:
                        nc.scalar.activation(relu[fq][:, fj, lsl], fp[:], AF.Relu)
                    else:
                        nc.vector.tensor_scalar_max(relu[fq][:, fj, lsl], fp[:],
                                                    0.0)

        w2_pool = p6.enter_context(tc.tile_pool(name="w2", bufs=4))
        out_pool = p6.enter_context(tc.tile_pool(name="outsb", bufs=4))
        b2_pool = p6.enter_context(tc.tile_pool(name="b2p", bufs=1))
        b2col = [b2_pool.tile([P, 1], f32, name=f"b2c{co}", tag=f"b2c{co}")
                 for co in range(NCT)]
        for co in range(NCT):
            nc.sync.dma_start(b2col[co][:], d_b2[co].unsqueeze(1))
        for co in range(NCT):
            w2a = w2_pool.tile([P, NFQ, 2, P], f8, tag="w2t")
            nc.sync.dma_start(w2a[:], d_w2[0, co])
            w2b = w2_pool.tile([P, NFQ, 2, P], f8, tag="w2t")
            nc.sync.dma_start(w2b[:], d_w2[1, co])
            for h in range(2):
                lsl = slice(h * 512, (h + 1) * 512)
                fp = ffn_ps.tile([P, 512], f32, tag="fp2")
                for fq in range(NFQ):
                    nc.tensor.matmul(fp[:], w2a[:, fq], relu[fq][:, :, lsl],
                                     start=(fq == 0), stop=False,
                                     perf_mode=PM.DoubleRow)
                for fq in range(NFQ):
                    nc.tensor.matmul(fp[:], w2b[:, fq], relu[fq][:, :, lsl],
                                     start=False, stop=(fq == NFQ - 1),
                                     perf_mode=PM.DoubleRow)
                tmp = out_pool.tile([P, 512], f32, tag="tmp")
                nc.vector.tensor_mul(tmp[:], fp[:], a2[h][:])
                osb = out_pool.tile([P, 512], f32, tag="osb")
                nc.vector.scalar_tensor_tensor(osb[:], tmp[:], b2col[co][:],
                                               x2sb[co][:, lsl], OP.add, OP.add)
                nc.sync.dma_start(d_out[co][:, lsl], osb[:])
        p6.close()

    nc.compile()
    return nc


def _q8(v):
    return np.asarray(v, np.float32).astype(E4)


def _prep_inputs(x, Wq, Wk, Wv, Wproj, bproj, W1, b1, W2, b2, g1, beta1, g2,
                 beta2):
    f32 = np.float32
    scale = HS ** -0.5
    x = np.asarray(x, f32)
    Wq = np.asarray(Wq, f32); Wk = np.asarray(Wk, f32); Wv = np.asarray(Wv, f32)
    Wproj = np.asarray(Wproj, f32); bproj = np.asarray(bproj, f32)
    W1 = np.asarray(W1, f32); b1 = np.asarray(b1, f32)
    W2 = np.asarray(W2, f32); b2 = np.asarray(b2, f32)
    g1 = np.asarray(g1, f32); beta1 = np.asarray(beta1, f32)
    g2 = np.asarray(g2, f32); beta2 = np.asarray(beta2, f32)

    # ---- shared (g-independent) weights ----
    w1s = (g2[:, None] * W1) * 32.0                       # [C, F]
    w1a = _q8(w1s)
    w1b = _q8(w1s - w1a.astype(f32))
    w1_pack = np.stack([w1a.reshape(NCP, 2, P, F).transpose(0, 2, 1, 3),
                        w1b.reshape(NCP, 2, P, F).transpose(0, 2, 1, 3)])
    cc1 = np.empty((2, F), f32)
    cc1[0] = (w1a.astype(f32) + w1b.astype(f32)).sum(0)
    cc1[1] = (b1 + beta2 @ W1) * 32.0
    cc1 = _q8(cc1)

    w2s = W2 * 32.0                                        # [F, C]
    w2a = _q8(w2s)
    w2b = _q8(w2s - w2a.astype(f32))
    # [2, NCT, P, NFQ, 2, P]: w2[s, co, p, fq, j, cc] = w2s[(2fq+j)*128+p, co*128+cc]
    def pack_w2(w):
        return np.ascontiguousarray(
            w.reshape(NFQ, 2, P, NCT, P).transpose(3, 2, 0, 1, 4))
    w2_pack = np.stack([pack_w2(w2a), pack_w2(w2b)])

    b2r = b2.reshape(NCT, P)
    kp = np.arange(P)[:, None]
    lq = np.arange(P)[None, :]
    trilneg = (-240.0 * (lq < kp)).astype(E4)
    ident20 = (20.0 * np.eye(P)).astype(E4)

    # ---- per-batch x ----
    xa_b, xsq_b, xresT_b = [], [], []
    for b in range(B):
        xT = np.ascontiguousarray(x[b].T)                  # [C, T]
        xq = _q8(xT)
        xa_b.append(np.ascontiguousarray(
            xq.reshape(NCP, 2, P, T).transpose(2, 0, 1, 3)))
        xsq_b.append(np.ascontiguousarray(
            _q8(xq.astype(f32) ** 2).reshape(NCP, 2, P, T).transpose(2, 0, 1, 3)))
        xresT_b.append(xT + bproj[:, None])                # bproj folded in

    # ---- per-group attention weights ----
    per_g = {}
    for g in range(2):
        wqk = np.empty((NPAIR, P, NCP, 2, 256), E4)
        ccqk = np.empty((NPAIR, 2, 256), f32)
        wv_ = np.empty((NPAIR, P, NCP, 2, P), E4)
        ccv = np.empty((NPAIR, 2, P), f32)
        for p in range(NPAIR):
            hA, hB = g * 8 + 2 * p, g * 8 + 2 * p + 1
            for (Wfull, scl, col) in ((Wq, scale * 16.0, slice(0, P)),
                                      (Wk, 32.0, slice(P, 256))):
                wt = np.concatenate([Wfull[hA], Wfull[hB]], axis=1) * scl  # [C,128]
                wq8 = _q8(g1[:, None] * wt)
                wqk[p, :, :, :, col] = wq8.reshape(NCP, 2, P, P).transpose(
                    2, 0, 1, 3)
                ccqk[p, 0, col] = wq8.astype(f32).sum(0)
                ccqk[p, 1, col] = beta1 @ wt
            wt = np.concatenate([Wv[hA], Wv[hB]], axis=1) * 32.0
            wq8 = _q8(g1[:, None] * wt)
            wv_[p] = wq8.reshape(NCP, 2, P, P).transpose(2, 0, 1, 3)
            ccv[p, 0] = wq8.astype(f32).sum(0)
            ccv[p, 1] = beta1 @ wt
        # wpj [2(split), 2(pp), P, 2(j), C]: wps[pp, j, f, c] -> [pp, f, j, c]
        wps = np.ascontiguousarray(
            Wproj[g * 512:(g + 1) * 512] * 32.0).reshape(2, 2, P, C)
        wpa = _q8(wps)
        wpb = _q8(wps - wpa.astype(f32))
        wpj = np.stack([wpa.transpose(0, 2, 1, 3), wpb.transpose(0, 2, 1, 3)])
        per_g[g] = dict(wqk=np.ascontiguousarray(wqk.transpose(1, 0, 2, 3, 4)),
                        ccqk=_q8(ccqk),
                        wv=np.ascontiguousarray(wv_.transpose(1, 0, 2, 3, 4)),
                        ccv=_q8(ccv),
                        wpj=np.ascontiguousarray(wpj.transpose(2, 0, 1, 3, 4)))

    in_maps = []
    for c in range(8):
        b, g = c // 2, c % 2
        m = dict(
            xa=xa_b[b], xsq=xsq_b[b],
            xres=np.ascontiguousarray(
                xresT_b[b][:, g * TL:(g + 1) * TL].reshape(NCT, P, TL)),
            w1=w1_pack, cc1=cc1, w2=w2_pack, b2=b2r,
            trilneg=trilneg, ident20=ident20,
        )
        m.update(per_g[g])
        in_maps.append(m)
    return in_maps


def kernel(**inputs):
    from concourse.bass_utils import run_bass_kernel_spmd

    if "nc" not in _CACHE:
        _CACHE["nc"] = _build(with_collective=True)
    nc = _CACHE["nc"]
    in_maps = _prep_inputs(**inputs)
    res = None
    last_err = None
    for _attempt in range(3):
        try:
            res = run_bass_kernel_spmd(nc, in_maps, list(range(8)))
            break
        except Exception as e:  # transient runtime/tunnel hiccups
            last_err = e
            import time
            time.sleep(10)
    if res is None:
        raise last_err
    out = np.empty((B, T, C), np.float32)
    for c in range(8):
        b, g = c // 2, c % 2
        outT = res.results[c]["outT"].reshape(C, TL)
        out[b, g * TL:(g + 1) * TL, :] = outT.T
    return out
